# revision 1
# baseline (speedup 1.0000x reference)
"""BRPConvEmbedding (3-layer GraphConv + AvgPool readout) on 8 Trainium2 cores.

Sharding: graphs are split contiguously across cores (32 graphs/core), so
pooling is core-local and the output is a pure concat. Each core owns the
nodes of its graphs; within a core, nodes are permuted into dst-groups of 64
nodes whose total in-degree per src-half is capped at 512 (4 chunks of 128
edge slots) via greedy bin-packing, which makes the per-group edge-chunk
layout uniform across all cores (single SPMD program).

Per layer: hn rows are fetched with SWDGE dma_gather (int16 indices; the node
table is split into two halves so indices fit in int16), the per-edge one-hot
is built on the VectorE (iota + tensor_tensor is_equal), the segment-sum runs
on the TensorE (lhsT=gathered chunk, rhs=onehot, PSUM accumulation), followed
by agg.T @ W + fused epilogue, and an AllGather of the new node features.
"""
import numpy as np
from contextlib import ExitStack

import concourse.bacc as bacc
import concourse.mybir as mybir
from concourse import tile
from concourse.bass_utils import run_bass_kernel_spmd

N_NODES = 50000
N_EDGES = 800000
D = 128
N_LAYERS = 3
N_GRAPHS = 256
NCORES = 8
GSZ = 64                  # dst nodes per group
CHUNKS_PER_HALF = 4       # 4*128 = 512 edge-slot cap per (group, half)
CAP = CHUNKS_PER_HALF * 128
GPC = N_GRAPHS // NCORES  # graphs per core


# ----------------------------------------------------------------- host prep
def _pack_groups(nodes, dA, dB):
    """Greedy bin-packing of nodes into groups of <= GSZ nodes with
    sum(dA) <= CAP and sum(dB) <= CAP per group. Returns group id per node."""
    order = np.argsort(-np.maximum(dA, dB), kind="stable")
    gids = np.full(len(nodes), -1, dtype=np.int64)
    usedA, usedB, usedN = [], [], []
    for i in order:
        a, b = dA[i], dB[i]
        best, best_fit = -1, -1.0
        for g in range(len(usedA)):
            if usedN[g] < GSZ and usedA[g] + a <= CAP and usedB[g] + b <= CAP:
                # best-fit: prefer the fullest group that still fits
                fit = max((usedA[g] + a) / CAP, (usedB[g] + b) / CAP)
                if fit > best_fit:
                    best, best_fit = g, fit
        if best < 0:
            usedA.append(0), usedB.append(0), usedN.append(0)
            best = len(usedA) - 1
        gids[i] = best
        usedA[best] += a
        usedB[best] += b
        usedN[best] += 1
    return gids, len(usedA)


def preprocess(feats, W, b, src, dst, graph_ids):
    src = np.asarray(src).astype(np.int64)
    dst = np.asarray(dst).astype(np.int64)
    graph_ids = np.asarray(graph_ids).astype(np.int64)
    feats = np.asarray(feats, dtype=np.float32)

    deg_out = np.maximum(np.bincount(src, minlength=N_NODES), 1).astype(np.float32)
    deg_in = np.maximum(np.bincount(dst, minlength=N_NODES), 1).astype(np.float32)

    node_core = graph_ids // GPC                      # node -> core
    src_half = (node_core[src] >= NCORES // 2).astype(np.int64)
    dA = np.bincount(dst[src_half == 0], minlength=N_NODES)
    dB = np.bincount(dst[src_half == 1], minlength=N_NODES)

    # pack nodes into groups per core
    core_nodes = [np.nonzero(node_core == c)[0] for c in range(NCORES)]
    packs = []
    Gmax = 0
    for c in range(NCORES):
        n = core_nodes[c]
        g, ng = _pack_groups(n, dA[n], dB[n])
        packs.append(g)
        Gmax = max(Gmax, ng)
    G = -(-Gmax // 4) * 4                             # multiple of 4 (supers of 4 groups)
    P = G // 2                                        # pairs (128-node tiles)
    NSUP = G // 4
    SH = G * GSZ                                      # rows per core shard
    R_half = (NCORES // 2) * SH
    assert R_half <= 32767, f"int16 overflow: {R_half}"

    # node -> row
    row = np.full(N_NODES, -1, dtype=np.int64)
    slot_in_group = np.zeros(N_NODES, dtype=np.int64)
    for c in range(NCORES):
        n = core_nodes[c]
        g = packs[c]
        order = np.lexsort((n, g))                    # stable by group
        n_sorted, g_sorted = n[order], g[order]
        # slot = rank within group
        slot = np.zeros(len(n), dtype=np.int64)
        _, starts = np.unique(g_sorted, return_index=True)
        for s0, s1 in zip(starts, list(starts[1:]) + [len(n)]):
            slot[s0:s1] = np.arange(s1 - s0)
        row[n_sorted] = c * SH + g_sorted * GSZ + slot
        slot_in_group[n_sorted] = slot

    # global row map for gather indices: 4 blocks (core-group x pair-half)
    # row_g(c, loc) = (c//4)*R_half + q*(R_half//2) + (c%4)*(SH//2) + (loc - q*(SH//2))
    # where q = loc >= SH//2
    loc_all = row - node_core * SH          # local row within shard (valid where row>=0)
    qh = (loc_all >= SH // 2).astype(np.int64)
    row_g = ((node_core // 4) * R_half + qh * (R_half // 2)
             + (node_core % 4) * (SH // 2) + (loc_all - qh * (SH // 2)))

    # per-core edge layout
    e_core = node_core[dst]
    e_group = np.zeros(N_EDGES, dtype=np.int64)
    for c in range(NCORES):
        m = e_core == c
        d_local = dst[m]
        lr = row[d_local] - c * SH
        e_group[m] = lr // GSZ
    e_dslot = (row[dst] % SH) % GSZ
    e_srow = row_g[src] - src_half * R_half           # int16-safe source row

    per_core = []
    for c in range(NCORES):
        m = np.nonzero(e_core == c)[0]
        g, h, sr, dslt = e_group[m], src_half[m], e_srow[m], e_dslot[m]
        order = np.lexsort((sr, h, g))
        g, h, sr, dslt = g[order], h[order], sr[order], dslt[order]
        # rank within (g, h)
        key = g * 2 + h
        rank = np.arange(len(m)) - np.searchsorted(key, key, side="left")
        k = rank // 128                               # chunk within (g,h)
        p = rank % 128
        assert (k < CHUNKS_PER_HALF).all(), "cap exceeded"
        gi = g % 4                                    # group idx in super
        s = g // 4
        c16 = gi * CHUNKS_PER_HALF + k                # chunk col within (super, half)
        j = c16 * 128 + p                             # slot within (super, half)

        # idx arrays [2*NSUP, 16, 128] (then tiled to 128 partitions)
        idx16 = np.zeros((2 * NSUP, 16, 128), dtype=np.int16)
        t = s * 2 + h
        idx16[t, j % 16, j // 16] = sr.astype(np.int16)
        idx_all = np.tile(idx16, (1, 8, 1)).reshape(2 * NSUP, 128, 128)
        idx_2d = idx_all.transpose(1, 0, 2).reshape(128, 2 * NSUP * 128).copy()

        # dst one-hot scalars [128, 2*NSUP*16], -1 for pad slots
        dstv = np.full((128, 2 * NSUP * 16), -1.0, dtype=np.float32)
        dstv[j % 128, t * 16 + c16] = dslt.astype(np.float32)

        # per-pair node scalars [128, P]
        nodes_c = core_nodes[c]
        lr = row[nodes_c] - c * SH
        deg_in_t = np.ones((128, P), dtype=np.float32)
        deg_out_t = np.ones((128, P), dtype=np.float32)
        gid_t = np.full((128, P), -1.0, dtype=np.float32)
        pr = lr // 128
        pp = lr % 128
        deg_in_t[pp, pr] = deg_in[nodes_c]
        deg_out_t[pp, pr] = deg_out[nodes_c]
        gid_t[pp, pr] = (graph_ids[nodes_c] - c * GPC).astype(np.float32)

        counts = np.maximum(
            np.bincount(graph_ids[nodes_c] - c * GPC, minlength=GPC), 1
        ).astype(np.float32).reshape(GPC, 1)

        feats_shard = np.zeros((SH, D), dtype=np.float32)
        feats_shard[lr] = feats[nodes_c]

        per_core.append(dict(
            idx=idx_2d, dstv=dstv, deg_in=deg_in_t, deg_out=deg_out_t,
            gid=gid_t, counts=counts, feats=feats_shard,
        ))

    b_rep = np.broadcast_to(
        np.asarray(b, dtype=np.float32)[None, :, :], (128, N_LAYERS, D)
    ).copy()
    meta = dict(G=G, P=P, NSUP=NSUP, SH=SH, R_half=R_half)
    shared = dict(W=np.ascontiguousarray(np.asarray(W, dtype=np.float32).transpose(1, 0, 2)),
                  b_rep=b_rep,
                  scr=np.zeros((NCORES * SH, D), dtype=np.float32))
    return per_core, shared, meta


# ------------------------------------------------------------- device build
def build(meta, rep=1, no_coll=False, no_gather=False, split_gather=2):
    G, P, NSUP, SH = meta["G"], meta["P"], meta["NSUP"], meta["SH"]
    R_half = meta["R_half"]
    CH = CHUNKS_PER_HALF
    f32 = mybir.dt.float32

    nc = bacc.Bacc("TRN2", target_bir_lowering=False, debug=False,
                   num_devices=NCORES, dynamic_dma_scratch_size=16384)

    idx_t = nc.dram_tensor("idx", [128, 2 * NSUP * 128], mybir.dt.int16, kind="ExternalInput")
    dstv_t = nc.dram_tensor("dstv", [128, 2 * NSUP * 16], f32, kind="ExternalInput")
    degi_t = nc.dram_tensor("deg_in", [128, P], f32, kind="ExternalInput")
    dego_t = nc.dram_tensor("deg_out", [128, P], f32, kind="ExternalInput")
    gid_t = nc.dram_tensor("gid", [128, P], f32, kind="ExternalInput")
    cnt_t = nc.dram_tensor("counts", [GPC, 1], f32, kind="ExternalInput")
    feats_t = nc.dram_tensor("feats", [SH, D], f32, kind="ExternalInput")
    W_t = nc.dram_tensor("W", [128, N_LAYERS, D], f32, kind="ExternalInput")
    brep_t = nc.dram_tensor("b_rep", [128, N_LAYERS, D], f32, kind="ExternalInput")
    scr_t = [nc.dram_tensor(f"scr{i}", [NCORES * SH, D], f32, kind="ExternalInput")
             for i in range(2)]
    out_t = nc.dram_tensor("out", [GPC, D], f32, kind="ExternalOutput")

    HSH = SH // 2
    hn_part = [
        [nc.dram_tensor(f"hn_p{i}_{q}", [NCORES * HSH, D], f32,
                        kind="Internal", addr_space="Shared") for q in range(2)]
        for i in range(N_LAYERS)
    ]

    def ag_and_copy(nc, hn_shard, l):
        """AllGather hn_shard (split by pair-halves) into scr[l % 2]."""
        scr = scr_t[l % 2]
        for q in range(2):
            nc.gpsimd.collective_compute(
                "AllGather", mybir.AluOpType.bypass,
                replica_groups=[list(range(NCORES))],
                ins=[hn_shard[q * HSH:(q + 1) * HSH, :].opt()],
                outs=[hn_part[l][q].ap().opt()],
            )
            half_blk = (NCORES // 2) * HSH
            for cg in range(2):
                dst0 = cg * (NCORES // 2) * SH + q * half_blk
                nc.sync.dma_start(
                    scr.ap()[dst0:dst0 + half_blk, :],
                    hn_part[l][q].ap()[cg * half_blk:(cg + 1) * half_blk, :],
                )

    with tile.TileContext(nc) as tc, ExitStack() as ctx:
        dram = ctx.enter_context(tc.tile_pool(name="dram", bufs=1, space="DRAM"))
        stat = ctx.enter_context(tc.tile_pool(name="stat", bufs=1))
        gpool = ctx.enter_context(tc.tile_pool(name="gath", bufs=4))
        opool = ctx.enter_context(tc.tile_pool(name="oh", bufs=4))
        spool = ctx.enter_context(tc.tile_pool(name="sb", bufs=4))
        ppool = ctx.enter_context(tc.tile_pool(name="agg_ps", bufs=4, space="PSUM"))
        hpool = ctx.enter_context(tc.tile_pool(name="h_ps", bufs=2, space="PSUM"))
        plpool = ctx.enter_context(tc.tile_pool(name="pool_ps", bufs=1, space="PSUM"))

        hn_shard = dram.tile([SH, D], f32)

        # ---- statics
        idx_sb = stat.tile([128, 2 * NSUP * 128], mybir.dt.int16)
        nc.sync.dma_start(idx_sb[:], idx_t.ap())
        dstv_sb = stat.tile([128, 2 * NSUP * 16], f32)
        nc.sync.dma_start(dstv_sb[:], dstv_t.ap())
        W_sb = stat.tile([128, N_LAYERS, D], f32)
        nc.sync.dma_start(W_sb[:], W_t.ap())
        brep_sb = stat.tile([128, N_LAYERS, D], f32)
        nc.sync.dma_start(brep_sb[:], brep_t.ap())
        gid_sb = stat.tile([128, P], f32)
        nc.sync.dma_start(gid_sb[:], gid_t.ap())
        cnt_sb = stat.tile([GPC, 1], f32)
        nc.sync.dma_start(cnt_sb[:], cnt_t.ap())

        degi_sb = stat.tile([128, P], f32)
        nc.sync.dma_start(degi_sb[:], degi_t.ap())
        dego_sb = stat.tile([128, P], f32)
        nc.sync.dma_start(dego_sb[:], dego_t.ap())
        ni_sb = stat.tile([128, P], f32)   # rsqrt(deg_in)
        no_sb = stat.tile([128, P], f32)   # rsqrt(deg_out)
        nc.vector.reciprocal(ni_sb[:], degi_sb[:])
        nc.scalar.activation(ni_sb[:], ni_sb[:], mybir.ActivationFunctionType.Sqrt)
        nc.vector.reciprocal(no_sb[:], dego_sb[:])
        nc.scalar.activation(no_sb[:], no_sb[:], mybir.ActivationFunctionType.Sqrt)
        rc_sb = stat.tile([GPC, 1], f32)   # 1/counts
        nc.vector.reciprocal(rc_sb[:], cnt_sb[:])

        iota16 = stat.tile([128, GSZ], mybir.dt.int16)
        nc.gpsimd.iota(iota16[:], pattern=[[1, GSZ]], base=0, channel_multiplier=0)
        iota_f = stat.tile([128, GSZ], f32)
        nc.vector.tensor_copy(iota_f[:], iota16[:])

        # graph one-hot [128, P, GPC] (built once; pooling uses layer-2 h)
        groh = stat.tile([128, P, GPC], f32)
        nc.vector.tensor_tensor(
            out=groh[:],
            in0=iota_f[:, :GPC].unsqueeze(1).broadcast_to([128, P, GPC]),
            in1=gid_sb[:].unsqueeze(2).broadcast_to([128, P, GPC]),
            op=mybir.AluOpType.is_equal,
        )

        for _ in range(rep):
            # ---- layer 0 input: hn0 = feats * norm_out
            for pr in range(P):
                ft = spool.tile([128, D], f32, tag="ft")
                nc.sync.dma_start(ft[:], feats_t.ap()[pr * 128:(pr + 1) * 128, :])
                hn0 = spool.tile([128, D], f32, tag="hn")
                nc.vector.tensor_scalar_mul(hn0[:], ft[:], no_sb[:, pr:pr + 1])
                nc.sync.dma_start(hn_shard[pr * 128:(pr + 1) * 128, :], hn0[:])
            if not no_coll:
                ag_and_copy(nc, hn_shard, 0)

            pool_ps = plpool.tile([GPC, D], f32)

            for l in range(N_LAYERS):
                for s in range(NSUP):
                    gA = gpool.tile([128, 4 * CH, D], f32, tag="gA")
                    gB = gpool.tile([128, 4 * CH, D], f32, tag="gB")
                    if split_gather > 1:
                        NP = split_gather          # pieces per half
                        NH = 4 * CH * 128 // NP
                        CPP = 4 * CH // NP         # chunks per piece
                        SCOL = 128 // NP           # idx cols per piece
                        for hh, gt in ((0, gA), (1, gB)):
                            base = scr_t[l % 2].ap()[0:R_half, :] if hh == 0 \
                                else scr_t[l % 2].ap()[R_half:, :]
                            for piece in range(NP):
                                nc.gpsimd.dma_gather(
                                    out_ap=gt[:, piece * CPP:(piece + 1) * CPP, :],
                                    in_ap=base,
                                    idxs_ap=idx_sb[:, (2 * s + hh) * 128 + piece * SCOL:
                                                   (2 * s + hh) * 128 + (piece + 1) * SCOL],
                                    num_idxs=NH, num_idxs_reg=NH,
                                    elem_size=D, single_packet=False)
                    elif not no_gather:
                        nc.gpsimd.dma_gather(
                            out_ap=gA[:], in_ap=scr_t[l % 2].ap()[0:R_half, :],
                            idxs_ap=idx_sb[:, (2 * s) * 128:(2 * s + 1) * 128],
                            num_idxs=4 * CH * 128, num_idxs_reg=4 * CH * 128,
                            elem_size=D, single_packet=False,
                        )
                        nc.gpsimd.dma_gather(
                            out_ap=gB[:], in_ap=scr_t[l % 2].ap()[R_half:, :],
                            idxs_ap=idx_sb[:, (2 * s + 1) * 128:(2 * s + 2) * 128],
                            num_idxs=4 * CH * 128, num_idxs_reg=4 * CH * 128,
                            elem_size=D, single_packet=False,
                        )
                    ohA = opool.tile([128, 4 * CH, GSZ], f32, tag="ohA")
                    ohB = opool.tile([128, 4 * CH, GSZ], f32, tag="ohB")
                    nc.vector.tensor_tensor(
                        out=ohA[:],
                        in0=iota_f[:].unsqueeze(1).broadcast_to([128, 4 * CH, GSZ]),
                        in1=dstv_sb[:, (2 * s) * 16:(2 * s) * 16 + 16]
                            .unsqueeze(2).broadcast_to([128, 4 * CH, GSZ]),
                        op=mybir.AluOpType.is_equal,
                    )
                    nc.vector.tensor_tensor(
                        out=ohB[:],
                        in0=iota_f[:].unsqueeze(1).broadcast_to([128, 4 * CH, GSZ]),
                        in1=dstv_sb[:, (2 * s + 1) * 16:(2 * s + 1) * 16 + 16]
                            .unsqueeze(2).broadcast_to([128, 4 * CH, GSZ]),
                        op=mybir.AluOpType.is_equal,
                    )
                    for pi in range(2):         # pairs in super
                        pr = s * 2 + pi
                        agg = ppool.tile([128, 128], f32, tag="agg")
                        for gj in range(2):     # groups in pair
                            gi = pi * 2 + gj
                            off = gj * GSZ
                            for k in range(CH):
                                nc.tensor.matmul(
                                    agg[:, off:off + GSZ],
                                    gA[:, gi * CH + k, :],
                                    ohA[:, gi * CH + k, :],
                                    start=(k == 0), stop=False,
                                    skip_group_check=True,
                                )
                            for k in range(CH):
                                nc.tensor.matmul(
                                    agg[:, off:off + GSZ],
                                    gB[:, gi * CH + k, :],
                                    ohB[:, gi * CH + k, :],
                                    start=False, stop=(k == CH - 1),
                                    skip_group_check=True,
                                )
                        agg_sb = spool.tile([128, 128], f32, tag="aggsb")
                        nc.scalar.copy(agg_sb[:], agg[:])
                        hps = hpool.tile([128, D], f32, tag="hps")
                        nc.tensor.matmul(hps[:], agg_sb[:], W_sb[:, l, :],
                                         start=True, stop=True)
                        t_sb = spool.tile([128, D], f32, tag="tsb")
                        nc.vector.scalar_tensor_tensor(
                            out=t_sb[:], in0=hps[:], scalar=ni_sb[:, pr:pr + 1],
                            in1=brep_sb[:, l, :],
                            op0=mybir.AluOpType.mult, op1=mybir.AluOpType.add,
                        )
                        if l < N_LAYERS - 1:
                            hn = spool.tile([128, D], f32, tag="hn2")
                            nc.vector.tensor_scalar(
                                out=hn[:], in0=t_sb[:],
                                scalar1=0.0, scalar2=no_sb[:, pr:pr + 1],
                                op0=mybir.AluOpType.max, op1=mybir.AluOpType.mult,
                            )
                            nc.sync.dma_start(
                                hn_shard[pr * 128:(pr + 1) * 128, :], hn[:])
                        else:
                            h_sb = spool.tile([128, D], f32, tag="hsb")
                            nc.vector.tensor_scalar_max(h_sb[:], t_sb[:], 0.0)
                            nc.tensor.matmul(
                                pool_ps[:], groh[:, pr, :], h_sb[:],
                                start=(pr == 0), stop=(pr == P - 1),
                            )
                if l < N_LAYERS - 1 and not no_coll:
                    ag_and_copy(nc, hn_shard, l + 1)

            pool_sb = spool.tile([GPC, D], f32, tag="poolsb")
            nc.vector.tensor_scalar_mul(pool_sb[:], pool_ps[:], rc_sb[:])
            nc.sync.dma_start(out_t.ap(), pool_sb[:])

    nc.compile()
    return nc


def make_in_maps(per_core, shared):
    in_maps = []
    for c in range(NCORES):
        pc = per_core[c]
        in_maps.append({
            "idx": pc["idx"], "dstv": pc["dstv"], "deg_in": pc["deg_in"],
            "deg_out": pc["deg_out"], "gid": pc["gid"], "counts": pc["counts"],
            "feats": pc["feats"], "W": shared["W"], "b_rep": shared["b_rep"],
            "scr0": shared["scr"], "scr1": shared["scr"],
        })
    return in_maps


def kernel(**inputs) -> np.ndarray:
    per_core, shared, meta = preprocess(**inputs)
    nc = build(meta, rep=1)
    in_maps = make_in_maps(per_core, shared)
    res = run_bass_kernel_spmd(nc, in_maps, core_ids=list(range(NCORES)))
    return np.concatenate([res.results[c]["out"] for c in range(NCORES)], axis=0)



# revision 41
# speedup vs baseline: 354.6183x; 354.6183x over previous
"""BRPConvEmbedding (3-layer GraphConv + AvgPool readout) on 8 Trainium2 cores.

Sharding: graphs are split contiguously across cores (32 graphs/core), so
pooling is core-local and the output is a pure concat. Each core owns the
nodes of its graphs; nodes are bin-packed into dst-groups of 64 nodes
whose in-degree per src core-half is capped at 512 (4 chunks of 128 edge
slots), making the per-group edge-chunk layout uniform across cores
(single SPMD program).

Per layer: ONE whole-shard AllGather of the bf16 node features; its
rank-major output [8*SH, D] is sliced at row 4*SH into the two
int16-addressable dma_gather bases (no HBM re-layout copy). Edge rows are
fetched with coarse SWDGE dma_gather calls (SB supers per call to
amortize Q7 descriptor generation), the per-edge one-hot is built on
VectorE in the int16 2x mode, the segment-sum runs on TensorE (bf16
lhsT = gathered chunk, rhs = one-hot, PSUM fp32 accumulation), followed
by agg.T @ W (bf16) + fused epilogue and a per-graph pooling matmul.
"""
import numpy as np
from contextlib import ExitStack

import concourse.bacc as bacc
import concourse.mybir as mybir
from concourse import tile
from concourse.bass_utils import run_bass_kernel_spmd

N_NODES = 50000
N_EDGES = 800000
D = 128
N_LAYERS = 3
N_GRAPHS = 256
NCORES = 8
GSZ = 64                  # dst nodes per group
CH = 4                    # chunks per (group, src-class): 4*128 = 512 cap
CAP = CH * 128
GPC = N_GRAPHS // NCORES  # graphs per core
SB = 4                    # supers per dma_gather call

BF16 = None  # set in preprocess via mybir.dt.np


# ----------------------------------------------------------------- host prep
def _pack_once(dA, dB, order):
    """Greedy best-fit packing of nodes into groups of <= GSZ nodes with
    sum(dA) <= CAP and sum(dB) <= CAP per group. Returns group id per node."""
    n = len(dA)
    gids = np.full(n, -1, dtype=np.int64)
    cap = 4 * (-(-n // GSZ)) + 64
    usedA = np.zeros(cap, dtype=np.int64)
    usedB = np.zeros(cap, dtype=np.int64)
    usedN = np.zeros(cap, dtype=np.int64)
    ng = 0
    for i in order:
        a, b = dA[i], dB[i]
        ok = ((usedN[:ng] < GSZ) & (usedA[:ng] + a <= CAP)
              & (usedB[:ng] + b <= CAP))
        if ok.any():
            fit = np.maximum(usedA[:ng] + a, usedB[:ng] + b)
            fit[~ok] = -1
            best = int(np.argmax(fit))
        else:
            best = ng
            ng += 1
        gids[i] = best
        usedA[best] += a
        usedB[best] += b
        usedN[best] += 1
    return gids, ng


def _pack_groups(dA, dB):
    """Multi-start best-fit packing; keeps the attempt with fewest groups."""
    base = np.argsort(-np.maximum(dA, dB), kind="stable")
    best = _pack_once(dA, dB, base)
    for seed in range(2):
        rng = np.random.default_rng(seed)
        key = np.maximum(dA, dB) + rng.uniform(0.0, 1.0, len(dA))
        g, ng = _pack_once(dA, dB, np.argsort(-key, kind="stable"))
        if ng < best[1]:
            best = (g, ng)
    return best


def preprocess(feats, W, b, src, dst, graph_ids):
    bf16 = mybir.dt.np(mybir.dt.bfloat16)
    src = np.asarray(src).astype(np.int64)
    dst = np.asarray(dst).astype(np.int64)
    graph_ids = np.asarray(graph_ids).astype(np.int64)
    feats = np.asarray(feats, dtype=np.float32)

    deg_out = np.maximum(np.bincount(src, minlength=N_NODES), 1).astype(np.float32)
    deg_in = np.maximum(np.bincount(dst, minlength=N_NODES), 1).astype(np.float32)

    # balance graphs across cores by node count (largest-first greedy);
    # kernel() undoes the permutation when assembling the output
    gsizes = np.bincount(graph_ids, minlength=N_GRAPHS)
    core_of_graph = np.zeros(N_GRAPHS, dtype=np.int64)
    loads = np.zeros(NCORES, dtype=np.int64)
    counts = np.zeros(NCORES, dtype=np.int64)
    for g in np.argsort(-gsizes, kind="stable"):
        elig = np.nonzero(counts < GPC)[0]
        c = elig[np.argmin(loads[elig])]
        core_of_graph[g] = c
        loads[c] += gsizes[g]
        counts[c] += 1
    # local graph slot within its core, in ascending global-graph order
    core_graphs = [np.nonzero(core_of_graph == c)[0] for c in range(NCORES)]
    glocal = np.zeros(N_GRAPHS, dtype=np.int64)
    for c in range(NCORES):
        glocal[core_graphs[c]] = np.arange(GPC)

    node_core = core_of_graph[graph_ids]
    core_nodes = [np.nonzero(node_core == c)[0] for c in range(NCORES)]

    # src classes = core halves (the two int16-addressable slices of the
    # whole-shard AllGather output)
    cls = (node_core >= NCORES // 2).astype(np.int64)
    sc = cls[src]
    dA = np.bincount(dst[sc == 0], minlength=N_NODES)
    dB = np.bincount(dst[sc == 1], minlength=N_NODES)

    # pack per core
    packs = {}
    gmax = 0
    for c in range(NCORES):
        n = core_nodes[c]
        g, ng = _pack_groups(dA[n], dB[n])
        packs[c] = (n, g)
        gmax = max(gmax, ng)
    G = -(-gmax // 4) * 4             # groups per shard (mult of 4 = supers)
    SH = G * GSZ                      # rows per core shard
    NSUP = G // 4
    P = G // 2
    R = (NCORES // 2) * SH            # rows per gather base (half the cores)
    assert R <= 32767, f"int16 overflow: {R}"

    # node -> row
    row = np.full(N_NODES, -1, dtype=np.int64)
    for c in range(NCORES):
        n, g = packs[c]
        order = np.lexsort((n, g))
        n_sorted, g_sorted = n[order], g[order]
        slot = np.zeros(len(n), dtype=np.int64)
        _, starts = np.unique(g_sorted, return_index=True)
        for s0, s1 in zip(starts, list(starts[1:]) + [len(n)]):
            slot[s0:s1] = np.arange(s1 - s0)
        row[n_sorted] = c * SH + g_sorted * GSZ + slot

    # gather index into base class q = row within that core-half's block
    gidx = row - cls * R
    assert gidx.min() >= 0 and gidx.max() < R

    # per-core edge layout
    e_core = node_core[dst]
    e_group = (row[dst] - e_core * SH) // GSZ          # 0..G-1 within shard
    e_dslot = (row[dst] - e_core * SH) % GSZ
    e_h = cls[src]
    e_sr = gidx[src]

    per_core = []
    for c in range(NCORES):
        m = np.nonzero(e_core == c)[0]
        g, h, sr, dslt = e_group[m], e_h[m], e_sr[m], e_dslot[m]
        order = np.lexsort((sr, h, g))
        g, h, sr, dslt = g[order], h[order], sr[order], dslt[order]
        key = g * 2 + h
        rank = np.arange(len(m)) - np.searchsorted(key, key, side="left")
        k = rank // 128                                # chunk within (g, h)
        p = rank % 128
        assert (k < CH).all(), "cap exceeded"
        gi = g % 4                                     # group in super
        s = g // 4                                     # super 0..NSUP-1
        c16 = gi * CH + k                              # chunk col in (super, class)
        j = c16 * 128 + p                              # slot in (super, class)

        # idx arrays, class-major: t = h*NSUP + s
        t = h * NSUP + s
        idx16 = np.zeros((2 * NSUP, 16, 128), dtype=np.int16)
        idx16[t, j % 16, j // 16] = sr.astype(np.int16)
        idx_all = np.tile(idx16, (1, 8, 1)).reshape(2 * NSUP, 128, 128)
        idx_2d = idx_all.transpose(1, 0, 2).reshape(128, 2 * NSUP * 128).copy()

        # dst one-hot scalars [128, 2*NSUP*16], -1 for pad slots (int16 so
        # the one-hot is_equal runs in the DVE 16-bit 2x mode)
        dstv = np.full((128, 2 * NSUP * 16), -1, dtype=np.int16)
        dstv[j % 128, t * 16 + c16] = dslt.astype(np.int16)

        # per-pair node scalars [128, P]
        nodes_c = core_nodes[c]
        lr = row[nodes_c] - c * SH
        ni_t = np.ones((128, P), dtype=np.float32)
        no_t = np.ones((128, P), dtype=np.float32)
        gid_t = np.full((128, P), -1.0, dtype=np.float32)
        pr = lr // 128
        pp = lr % 128
        ni_t[pp, pr] = 1.0 / np.sqrt(deg_in[nodes_c])
        no_t[pp, pr] = 1.0 / np.sqrt(deg_out[nodes_c])
        gid_t[pp, pr] = glocal[graph_ids[nodes_c]].astype(np.float32)

        rcnt = (1.0 / np.maximum(
            np.bincount(glocal[graph_ids[nodes_c]], minlength=GPC), 1
        )).astype(np.float32).reshape(GPC, 1)

        # layer-0 input precomputed on host: hn0 = feats * norm_out (bf16)
        h0 = np.zeros((SH, D), dtype=np.float32)
        h0[lr] = feats[nodes_c] / np.sqrt(deg_out[nodes_c])[:, None]
        h0 = h0.astype(bf16)

        per_core.append(dict(
            idx=idx_2d, dstv=dstv, ni=ni_t, no=no_t,
            gid=gid_t, rcnt=rcnt, h0=h0,
        ))

    b_rep = np.broadcast_to(
        np.asarray(b, dtype=np.float32)[None, :, :], (128, N_LAYERS, D)
    ).copy()
    W_bf = np.ascontiguousarray(
        np.asarray(W, dtype=np.float32).transpose(1, 0, 2)
    ).astype(bf16)
    meta = dict(G=G, P=P, NSUP=NSUP, SH=SH, R=R, core_graphs=core_graphs)
    shared = dict(W=W_bf, b_rep=b_rep)
    return per_core, shared, meta


# ------------------------------------------------------------- device build
def build(meta, rep=1, no_coll=False, no_gather=False, sp=False, nq=4,
          no_mm=False, no_oh=False, scratch=65536, sb=SB, gbufs=2,
          aggact=False):
    G, P, NSUP, SH, R = (meta["G"], meta["P"], meta["NSUP"],
                         meta["SH"], meta["R"])
    # gather blocks: sb supers per call (+ remainder block), 2 classes each
    blocks = [(b * sb, sb) for b in range(NSUP // sb)]
    if NSUP % sb:
        blocks.append((NSUP - NSUP % sb, NSUP % sb))
    f32 = mybir.dt.float32
    bf16 = mybir.dt.bfloat16
    i16 = mybir.dt.int16

    nc = bacc.Bacc("TRN2", target_bir_lowering=False, debug=False,
                   num_devices=NCORES, dynamic_dma_scratch_size=scratch,
                   num_swdge_queues=nq)

    idx_t = nc.dram_tensor("idx", [128, 2 * NSUP * 128], i16, kind="ExternalInput")
    dstv_t = nc.dram_tensor("dstv", [128, 2 * NSUP * 16], i16, kind="ExternalInput")
    ni_t = nc.dram_tensor("ni", [128, P], f32, kind="ExternalInput")
    no_t = nc.dram_tensor("no", [128, P], f32, kind="ExternalInput")
    gid_t = nc.dram_tensor("gid", [128, P], f32, kind="ExternalInput")
    rcnt_t = nc.dram_tensor("rcnt", [GPC, 1], f32, kind="ExternalInput")
    h0_t = nc.dram_tensor("h0", [SH, D], bf16, kind="ExternalInput")
    W_t = nc.dram_tensor("W", [128, N_LAYERS, D], bf16, kind="ExternalInput")
    brep_t = nc.dram_tensor("b_rep", [128, N_LAYERS, D], f32, kind="ExternalInput")
    out_t = nc.dram_tensor("out", [GPC, D], f32, kind="ExternalOutput")

    # Whole-shard AllGather output, double-buffered by layer parity. The
    # rank-major output [8*SH, D] is sliced into two int16-addressable
    # gather bases: rows [0, R) = cores 0-3, rows [R, 2R) = cores 4-7.
    hnall = [
        nc.dram_tensor(f"hnall{par}", [NCORES * SH, D], bf16,
                       kind="Internal", addr_space="Shared")
        for par in range(2)
    ]

    def ag(hn_shard, l):
        """AllGather the full shard into hnall[l % 2]."""
        if no_coll:
            return
        nc.gpsimd.collective_compute(
            "AllGather", mybir.AluOpType.bypass,
            replica_groups=[list(range(NCORES))],
            ins=[hn_shard[:].opt()],
            outs=[hnall[l % 2].ap().opt()],
        )

    with tile.TileContext(nc) as tc, ExitStack() as ctx:
        dram = ctx.enter_context(tc.tile_pool(name="dram", bufs=1, space="DRAM"))
        stat = ctx.enter_context(tc.tile_pool(name="stat", bufs=1))
        gpool = ctx.enter_context(tc.tile_pool(name="gath", bufs=gbufs))
        opool = ctx.enter_context(tc.tile_pool(name="oh", bufs=2))
        spool = ctx.enter_context(tc.tile_pool(name="sb", bufs=4))
        ppool = ctx.enter_context(tc.tile_pool(name="agg_ps", bufs=4, space="PSUM"))
        hpool = ctx.enter_context(tc.tile_pool(name="h_ps", bufs=2, space="PSUM"))
        plpool = ctx.enter_context(tc.tile_pool(name="pool_ps", bufs=1, space="PSUM"))

        hn_shard = dram.tile([SH, D], bf16)

        # ---- statics
        idx_sb = stat.tile([128, 2 * NSUP * 128], i16)
        nc.sync.dma_start(idx_sb[:], idx_t.ap())
        dstv_sb = stat.tile([128, 2 * NSUP * 16], i16)
        nc.sync.dma_start(dstv_sb[:], dstv_t.ap())
        W_sb = stat.tile([128, N_LAYERS, D], bf16)
        nc.sync.dma_start(W_sb[:], W_t.ap())
        brep_sb = stat.tile([128, N_LAYERS, D], f32)
        nc.sync.dma_start(brep_sb[:], brep_t.ap())
        gid_sb = stat.tile([128, P], f32)
        nc.sync.dma_start(gid_sb[:], gid_t.ap())
        rc_sb = stat.tile([GPC, 1], f32)
        nc.sync.dma_start(rc_sb[:], rcnt_t.ap())
        ni_sb = stat.tile([128, P], f32)
        nc.sync.dma_start(ni_sb[:], ni_t.ap())
        no_sb = stat.tile([128, P], f32)
        nc.sync.dma_start(no_sb[:], no_t.ap())

        iota16 = stat.tile([128, GSZ], mybir.dt.int16)
        nc.gpsimd.iota(iota16[:], pattern=[[1, GSZ]], base=0, channel_multiplier=0)
        iota_f = stat.tile([128, GSZ], f32)
        nc.vector.tensor_copy(iota_f[:], iota16[:])

        # graph one-hot [128, P, GPC] (pooling uses layer-2 h)
        groh = stat.tile([128, P, GPC], f32)
        nc.vector.tensor_tensor(
            out=groh[:],
            in0=iota_f[:, :GPC].unsqueeze(1).broadcast_to([128, P, GPC]),
            in1=gid_sb[:].unsqueeze(2).broadcast_to([128, P, GPC]),
            op=mybir.AluOpType.is_equal,
        )

        for _ in range(rep):
            # ---- layer 0 input: hn0 precomputed on host; stage + AllGather
            nc.sync.dma_start(hn_shard[:], h0_t.ap())
            ag(hn_shard, 0)

            pool_ps = plpool.tile([GPC, D], f32)

            for l in range(N_LAYERS):
                par = l % 2
                for bi, (s0, sbn) in enumerate(blocks):
                    gt = [None, None]
                    oht = [None, None]
                    for h in range(2):     # h = src class (gather base)
                        # padded to SB supers so the remainder block shares
                        # the same pool tag (same SBUF slots)
                        gt[h] = gpool.tile([128, sb * 16, D], bf16,
                                           tag=f"g{h}", name=f"g{h}")
                        col0 = (h * NSUP + s0) * 128
                        if not no_gather:
                            nc.gpsimd.dma_gather(
                                out_ap=gt[h][:, :sbn * 16, :],
                                in_ap=hnall[par].ap()[h * R:(h + 1) * R, :],
                                idxs_ap=idx_sb[:, col0:col0 + sbn * 128],
                                num_idxs=sbn * 16 * 128,
                                num_idxs_reg=sbn * 16 * 128,
                                elem_size=D, single_packet=sp,
                                queue_num=(2 * bi + h) % nq,
                            )
                        oht[h] = opool.tile([128, sb * 16, GSZ], bf16,
                                            tag=f"oh{h}", name=f"oh{h}")
                        sc0 = (h * NSUP + s0) * 16
                        if not no_oh:
                            nc.vector.tensor_tensor(
                                out=oht[h][:, :sbn * 16, :],
                                in0=iota16[:].unsqueeze(1).broadcast_to([128, sbn * 16, GSZ]),
                                in1=dstv_sb[:, sc0:sc0 + sbn * 16]
                                    .unsqueeze(2).broadcast_to([128, sbn * 16, GSZ]),
                                op=mybir.AluOpType.is_equal,
                            )
                    if no_mm:
                        continue
                    for si in range(sbn):
                        s = s0 + si                   # dst super 0..NSUP-1
                        for pi in range(2):           # pairs in super
                            pr = s * 2 + pi
                            agg = ppool.tile([128, 128], f32, tag="agg")
                            for gj in range(2):       # groups in pair
                                gi = pi * 2 + gj
                                off = gj * GSZ
                                for hh in range(2):   # src classes
                                    for k in range(CH):
                                        nc.tensor.matmul(
                                            agg[:, off:off + GSZ],
                                            gt[hh][:, si * 16 + gi * CH + k, :],
                                            oht[hh][:, si * 16 + gi * CH + k, :],
                                            start=(hh == 0 and k == 0),
                                            stop=(hh == 1 and k == CH - 1),
                                            skip_group_check=True,
                                        )
                            agg_sb = spool.tile([128, 128], bf16, tag="aggsb")
                            if aggact:
                                nc.scalar.copy(agg_sb[:], agg[:])
                            else:
                                nc.vector.tensor_copy(agg_sb[:], agg[:])
                            hps = hpool.tile([128, D], f32, tag="hps")
                            nc.tensor.matmul(hps[:], agg_sb[:], W_sb[:, l, :],
                                             start=True, stop=True)
                            t_sb = spool.tile([128, D], f32, tag="tsb")
                            nc.vector.scalar_tensor_tensor(
                                out=t_sb[:], in0=hps[:],
                                scalar=ni_sb[:, pr:pr + 1],
                                in1=brep_sb[:, l, :],
                                op0=mybir.AluOpType.mult,
                                op1=mybir.AluOpType.add,
                            )
                            if l < N_LAYERS - 1:
                                # hn = relu(t) * no == relu(t * no), no > 0;
                                # runs on the otherwise-idle ScalarE
                                hn = spool.tile([128, D], bf16, tag="hn2")
                                nc.scalar.activation(
                                    hn[:], t_sb[:],
                                    mybir.ActivationFunctionType.Relu,
                                    scale=no_sb[:, pr:pr + 1],
                                )
                                nc.sync.dma_start(
                                    hn_shard[pr * 128:(pr + 1) * 128, :], hn[:])
                                if pr == P - 1:
                                    ag(hn_shard, l + 1)
                            else:
                                h_sb = spool.tile([128, D], f32, tag="hsb")
                                nc.scalar.activation(
                                    h_sb[:], t_sb[:],
                                    mybir.ActivationFunctionType.Relu,
                                )
                                nc.tensor.matmul(
                                    pool_ps[:], groh[:, pr, :], h_sb[:],
                                    start=(pr == 0), stop=(pr == P - 1),
                                )

            pool_sb = spool.tile([GPC, D], f32, tag="poolsb")
            nc.vector.tensor_scalar_mul(pool_sb[:], pool_ps[:], rc_sb[:])
            nc.sync.dma_start(out_t.ap(), pool_sb[:])

    nc.compile()
    return nc


def make_in_maps(per_core, shared):
    in_maps = []
    for c in range(NCORES):
        pc = per_core[c]
        in_maps.append({
            "idx": pc["idx"], "dstv": pc["dstv"], "ni": pc["ni"],
            "no": pc["no"], "gid": pc["gid"], "rcnt": pc["rcnt"],
            "h0": pc["h0"], "W": shared["W"], "b_rep": shared["b_rep"],
        })
    return in_maps


def kernel(**inputs) -> np.ndarray:
    per_core, shared, meta = preprocess(**inputs)
    nc = build(meta, rep=1)
    in_maps = make_in_maps(per_core, shared)
    res = run_bass_kernel_spmd(nc, in_maps, core_ids=list(range(NCORES)))
    out = np.zeros((N_GRAPHS, D), dtype=np.float32)
    for c in range(NCORES):
        out[meta["core_graphs"][c]] = res.results[c]["out"]
    return out


# revision 42
# speedup vs baseline: 521.4870x; 1.4706x over previous
"""BRPConvEmbedding (3-layer GraphConv + AvgPool readout) on 8 Trainium2 cores.

Sharding: graphs are split contiguously across cores (32 graphs/core), so
pooling is core-local and the output is a pure concat. Each core owns the
nodes of its graphs; nodes are bin-packed into dst-groups of 64 nodes
whose in-degree per src core-half is capped at 512 (4 chunks of 128 edge
slots), making the per-group edge-chunk layout uniform across cores
(single SPMD program).

Per layer: ONE whole-shard AllGather of the bf16 node features; its
rank-major output [8*SH, D] is sliced at row 4*SH into the two
int16-addressable dma_gather bases (no HBM re-layout copy). Edge rows are
fetched with coarse SWDGE dma_gather calls (SB supers per call to
amortize Q7 descriptor generation), the per-edge one-hot is built on
VectorE in the int16 2x mode, the segment-sum runs on TensorE (bf16
lhsT = gathered chunk, rhs = one-hot, PSUM fp32 accumulation), followed
by agg.T @ W (bf16) + fused epilogue and a per-graph pooling matmul.
"""
import numpy as np
from contextlib import ExitStack

import concourse.bacc as bacc
import concourse.mybir as mybir
from concourse import tile
from concourse.bass_utils import run_bass_kernel_spmd

N_NODES = 50000
N_EDGES = 800000
D = 128
N_LAYERS = 3
N_GRAPHS = 256
NCORES = 8
GSZ = 64                  # dst nodes per group
CH = 4                    # chunks per (group, src-class): 4*128 = 512 cap
CAP = CH * 128
GPC = N_GRAPHS // NCORES  # graphs per core
SB = 4                    # supers per dma_gather call

BF16 = None  # set in preprocess via mybir.dt.np


# ----------------------------------------------------------------- host prep
def _pack_once(dA, dB, order):
    """Greedy best-fit packing of nodes into groups of <= GSZ nodes with
    sum(dA) <= CAP and sum(dB) <= CAP per group. Returns group id per node."""
    n = len(dA)
    gids = np.full(n, -1, dtype=np.int64)
    cap = 4 * (-(-n // GSZ)) + 64
    usedA = np.zeros(cap, dtype=np.int64)
    usedB = np.zeros(cap, dtype=np.int64)
    usedN = np.zeros(cap, dtype=np.int64)
    ng = 0
    for i in order:
        a, b = dA[i], dB[i]
        ok = ((usedN[:ng] < GSZ) & (usedA[:ng] + a <= CAP)
              & (usedB[:ng] + b <= CAP))
        if ok.any():
            fit = np.maximum(usedA[:ng] + a, usedB[:ng] + b)
            fit[~ok] = -1
            best = int(np.argmax(fit))
        else:
            best = ng
            ng += 1
        gids[i] = best
        usedA[best] += a
        usedB[best] += b
        usedN[best] += 1
    return gids, ng


def _pack_groups(dA, dB):
    """Multi-start best-fit packing; keeps the attempt with fewest groups."""
    base = np.argsort(-np.maximum(dA, dB), kind="stable")
    best = _pack_once(dA, dB, base)
    for seed in range(2):
        rng = np.random.default_rng(seed)
        key = np.maximum(dA, dB) + rng.uniform(0.0, 1.0, len(dA))
        g, ng = _pack_once(dA, dB, np.argsort(-key, kind="stable"))
        if ng < best[1]:
            best = (g, ng)
    return best


def preprocess(feats, W, b, src, dst, graph_ids):
    bf16 = mybir.dt.np(mybir.dt.bfloat16)
    src = np.asarray(src).astype(np.int64)
    dst = np.asarray(dst).astype(np.int64)
    graph_ids = np.asarray(graph_ids).astype(np.int64)
    feats = np.asarray(feats, dtype=np.float32)

    deg_out = np.maximum(np.bincount(src, minlength=N_NODES), 1).astype(np.float32)
    deg_in = np.maximum(np.bincount(dst, minlength=N_NODES), 1).astype(np.float32)

    # balance graphs across cores by node count (largest-first greedy);
    # kernel() undoes the permutation when assembling the output
    gsizes = np.bincount(graph_ids, minlength=N_GRAPHS)
    core_of_graph = np.zeros(N_GRAPHS, dtype=np.int64)
    loads = np.zeros(NCORES, dtype=np.int64)
    counts = np.zeros(NCORES, dtype=np.int64)
    for g in np.argsort(-gsizes, kind="stable"):
        elig = np.nonzero(counts < GPC)[0]
        c = elig[np.argmin(loads[elig])]
        core_of_graph[g] = c
        loads[c] += gsizes[g]
        counts[c] += 1
    # local graph slot within its core, in ascending global-graph order
    core_graphs = [np.nonzero(core_of_graph == c)[0] for c in range(NCORES)]
    glocal = np.zeros(N_GRAPHS, dtype=np.int64)
    for c in range(NCORES):
        glocal[core_graphs[c]] = np.arange(GPC)

    node_core = core_of_graph[graph_ids]
    core_nodes = [np.nonzero(node_core == c)[0] for c in range(NCORES)]

    # src classes = core halves (the two int16-addressable slices of the
    # whole-shard AllGather output)
    cls = (node_core >= NCORES // 2).astype(np.int64)
    sc = cls[src]
    dA = np.bincount(dst[sc == 0], minlength=N_NODES)
    dB = np.bincount(dst[sc == 1], minlength=N_NODES)

    # pack per core
    packs = {}
    gmax = 0
    for c in range(NCORES):
        n = core_nodes[c]
        g, ng = _pack_groups(dA[n], dB[n])
        packs[c] = (n, g)
        gmax = max(gmax, ng)
    G = -(-gmax // 4) * 4             # groups per shard (mult of 4 = supers)
    SH = G * GSZ                      # rows per core shard
    NSUP = G // 4
    P = G // 2
    R = (NCORES // 2) * SH            # rows per gather base (half the cores)
    assert R <= 32767, f"int16 overflow: {R}"

    # node -> row
    row = np.full(N_NODES, -1, dtype=np.int64)
    for c in range(NCORES):
        n, g = packs[c]
        order = np.lexsort((n, g))
        n_sorted, g_sorted = n[order], g[order]
        slot = np.zeros(len(n), dtype=np.int64)
        _, starts = np.unique(g_sorted, return_index=True)
        for s0, s1 in zip(starts, list(starts[1:]) + [len(n)]):
            slot[s0:s1] = np.arange(s1 - s0)
        row[n_sorted] = c * SH + g_sorted * GSZ + slot

    # gather index into base class q = row within that core-half's block
    gidx = row - cls * R
    assert gidx.min() >= 0 and gidx.max() < R

    # per-core edge layout
    e_core = node_core[dst]
    e_group = (row[dst] - e_core * SH) // GSZ          # 0..G-1 within shard
    e_dslot = (row[dst] - e_core * SH) % GSZ
    e_h = cls[src]
    e_sr = gidx[src]

    per_core = []
    for c in range(NCORES):
        m = np.nonzero(e_core == c)[0]
        g, h, sr, dslt = e_group[m], e_h[m], e_sr[m], e_dslot[m]
        order = np.lexsort((sr, h, g))
        g, h, sr, dslt = g[order], h[order], sr[order], dslt[order]
        key = g * 2 + h
        rank = np.arange(len(m)) - np.searchsorted(key, key, side="left")
        k = rank // 128                                # chunk within (g, h)
        p = rank % 128
        assert (k < CH).all(), "cap exceeded"
        gi = g % 4                                     # group in super
        s = g // 4                                     # super 0..NSUP-1
        c16 = gi * CH + k                              # chunk col in (super, class)
        j = c16 * 128 + p                              # slot in (super, class)

        # idx arrays, class-major: t = h*NSUP + s
        t = h * NSUP + s
        idx16 = np.zeros((2 * NSUP, 16, 128), dtype=np.int16)
        idx16[t, j % 16, j // 16] = sr.astype(np.int16)
        idx_all = np.tile(idx16, (1, 8, 1)).reshape(2 * NSUP, 128, 128)
        idx_2d = idx_all.transpose(1, 0, 2).reshape(128, 2 * NSUP * 128).copy()

        # dst one-hot scalars [128, 2*NSUP*16], -1 for pad slots (int16 so
        # the one-hot is_equal runs in the DVE 16-bit 2x mode)
        dstv = np.full((128, 2 * NSUP * 16), -1, dtype=np.int16)
        dstv[j % 128, t * 16 + c16] = dslt.astype(np.int16)

        # per-pair node scalars [128, P]
        nodes_c = core_nodes[c]
        lr = row[nodes_c] - c * SH
        ni_t = np.ones((128, P), dtype=np.float32)
        no_t = np.ones((128, P), dtype=np.float32)
        gid_t = np.full((128, P), -1.0, dtype=np.float32)
        pr = lr // 128
        pp = lr % 128
        ni_t[pp, pr] = 1.0 / np.sqrt(deg_in[nodes_c])
        no_t[pp, pr] = 1.0 / np.sqrt(deg_out[nodes_c])
        gid_t[pp, pr] = glocal[graph_ids[nodes_c]].astype(np.float32)

        rcnt = (1.0 / np.maximum(
            np.bincount(glocal[graph_ids[nodes_c]], minlength=GPC), 1
        )).astype(np.float32).reshape(GPC, 1)

        # layer-0 input precomputed on host: hn0 = feats * norm_out (bf16)
        h0 = np.zeros((SH, D), dtype=np.float32)
        h0[lr] = feats[nodes_c] / np.sqrt(deg_out[nodes_c])[:, None]
        h0 = h0.astype(bf16)

        per_core.append(dict(
            idx=idx_2d, dstv=dstv, ni=ni_t, no=no_t,
            gid=gid_t, rcnt=rcnt, h0=h0,
        ))

    b_rep = np.broadcast_to(
        np.asarray(b, dtype=np.float32)[None, :, :], (128, N_LAYERS, D)
    ).copy()
    W_bf = np.ascontiguousarray(
        np.asarray(W, dtype=np.float32).transpose(1, 0, 2)
    ).astype(bf16)
    meta = dict(G=G, P=P, NSUP=NSUP, SH=SH, R=R, core_graphs=core_graphs)
    shared = dict(W=W_bf, b_rep=b_rep)
    return per_core, shared, meta


# ------------------------------------------------------------- device build
def build(meta, rep=1, no_coll=False, no_gather=False, sp=False, nq=4,
          no_mm=False, no_oh=False, scratch=65536, sb=SB, gbufs=2,
          aggact=False, pbufs=4, sbufs=4):
    G, P, NSUP, SH, R = (meta["G"], meta["P"], meta["NSUP"],
                         meta["SH"], meta["R"])
    # gather blocks: sb supers per call (+ remainder block), 2 classes each
    blocks = [(b * sb, sb) for b in range(NSUP // sb)]
    if NSUP % sb:
        blocks.append((NSUP - NSUP % sb, NSUP % sb))
    f32 = mybir.dt.float32
    bf16 = mybir.dt.bfloat16
    i16 = mybir.dt.int16

    nc = bacc.Bacc("TRN2", target_bir_lowering=False, debug=False,
                   num_devices=NCORES, dynamic_dma_scratch_size=scratch,
                   num_swdge_queues=nq)

    idx_t = nc.dram_tensor("idx", [128, 2 * NSUP * 128], i16, kind="ExternalInput")
    dstv_t = nc.dram_tensor("dstv", [128, 2 * NSUP * 16], i16, kind="ExternalInput")
    ni_t = nc.dram_tensor("ni", [128, P], f32, kind="ExternalInput")
    no_t = nc.dram_tensor("no", [128, P], f32, kind="ExternalInput")
    gid_t = nc.dram_tensor("gid", [128, P], f32, kind="ExternalInput")
    rcnt_t = nc.dram_tensor("rcnt", [GPC, 1], f32, kind="ExternalInput")
    h0_t = nc.dram_tensor("h0", [SH, D], bf16, kind="ExternalInput")
    W_t = nc.dram_tensor("W", [128, N_LAYERS, D], bf16, kind="ExternalInput")
    brep_t = nc.dram_tensor("b_rep", [128, N_LAYERS, D], f32, kind="ExternalInput")
    out_t = nc.dram_tensor("out", [GPC, D], f32, kind="ExternalOutput")

    # Whole-shard AllGather output, double-buffered by layer parity. The
    # rank-major output [8*SH, D] is sliced into two int16-addressable
    # gather bases: rows [0, R) = cores 0-3, rows [R, 2R) = cores 4-7.
    hnall = [
        nc.dram_tensor(f"hnall{par}", [NCORES * SH, D], bf16,
                       kind="Internal", addr_space="Shared")
        for par in range(2)
    ]

    def ag(hn_shard, l):
        """AllGather the full shard into hnall[l % 2]."""
        if no_coll:
            return
        nc.gpsimd.collective_compute(
            "AllGather", mybir.AluOpType.bypass,
            replica_groups=[list(range(NCORES))],
            ins=[hn_shard[:].opt()],
            outs=[hnall[l % 2].ap().opt()],
        )

    with tile.TileContext(nc) as tc, ExitStack() as ctx:
        dram = ctx.enter_context(tc.tile_pool(name="dram", bufs=1, space="DRAM"))
        stat = ctx.enter_context(tc.tile_pool(name="stat", bufs=1))
        gpool = ctx.enter_context(tc.tile_pool(name="gath", bufs=gbufs))
        opool = ctx.enter_context(tc.tile_pool(name="oh", bufs=2))
        spool = ctx.enter_context(tc.tile_pool(name="sb", bufs=sbufs))
        ppool = ctx.enter_context(tc.tile_pool(name="agg_ps", bufs=pbufs, space="PSUM"))
        hpool = ctx.enter_context(tc.tile_pool(name="h_ps", bufs=2, space="PSUM"))
        plpool = ctx.enter_context(tc.tile_pool(name="pool_ps", bufs=1, space="PSUM"))

        hn_shard = dram.tile([SH, D], bf16)

        # ---- statics
        idx_sb = stat.tile([128, 2 * NSUP * 128], i16)
        nc.sync.dma_start(idx_sb[:], idx_t.ap())
        dstv_sb = stat.tile([128, 2 * NSUP * 16], i16)
        nc.sync.dma_start(dstv_sb[:], dstv_t.ap())
        W_sb = stat.tile([128, N_LAYERS, D], bf16)
        nc.sync.dma_start(W_sb[:], W_t.ap())
        brep_sb = stat.tile([128, N_LAYERS, D], f32)
        nc.sync.dma_start(brep_sb[:], brep_t.ap())
        gid_sb = stat.tile([128, P], f32)
        nc.sync.dma_start(gid_sb[:], gid_t.ap())
        rc_sb = stat.tile([GPC, 1], f32)
        nc.sync.dma_start(rc_sb[:], rcnt_t.ap())
        ni_sb = stat.tile([128, P], f32)
        nc.sync.dma_start(ni_sb[:], ni_t.ap())
        no_sb = stat.tile([128, P], f32)
        nc.sync.dma_start(no_sb[:], no_t.ap())

        iota16 = stat.tile([128, GSZ], mybir.dt.int16)
        nc.gpsimd.iota(iota16[:], pattern=[[1, GSZ]], base=0, channel_multiplier=0)
        iota_f = stat.tile([128, GSZ], f32)
        nc.vector.tensor_copy(iota_f[:], iota16[:])

        # graph one-hot [128, P, GPC] (pooling uses layer-2 h)
        groh = stat.tile([128, P, GPC], f32)
        nc.vector.tensor_tensor(
            out=groh[:],
            in0=iota_f[:, :GPC].unsqueeze(1).broadcast_to([128, P, GPC]),
            in1=gid_sb[:].unsqueeze(2).broadcast_to([128, P, GPC]),
            op=mybir.AluOpType.is_equal,
        )

        for _ in range(rep):
            # ---- layer 0 input: hn0 precomputed on host; stage + AllGather
            nc.sync.dma_start(hn_shard[:], h0_t.ap())
            ag(hn_shard, 0)

            pool_ps = plpool.tile([GPC, D], f32)

            for l in range(N_LAYERS):
                par = l % 2
                for bi, (s0, sbn) in enumerate(blocks):
                    gt = [None, None]
                    oht = [None, None]
                    for h in range(2):     # h = src class (gather base)
                        # padded to SB supers so the remainder block shares
                        # the same pool tag (same SBUF slots)
                        gt[h] = gpool.tile([128, sb * 16, D], bf16,
                                           tag=f"g{h}", name=f"g{h}")
                        col0 = (h * NSUP + s0) * 128
                        if not no_gather:
                            nc.gpsimd.dma_gather(
                                out_ap=gt[h][:, :sbn * 16, :],
                                in_ap=hnall[par].ap()[h * R:(h + 1) * R, :],
                                idxs_ap=idx_sb[:, col0:col0 + sbn * 128],
                                num_idxs=sbn * 16 * 128,
                                num_idxs_reg=sbn * 16 * 128,
                                elem_size=D, single_packet=sp,
                                queue_num=(2 * bi + h) % nq,
                            )
                        oht[h] = opool.tile([128, sb * 16, GSZ], bf16,
                                            tag=f"oh{h}", name=f"oh{h}")
                        sc0 = (h * NSUP + s0) * 16
                        if not no_oh:
                            nc.vector.tensor_tensor(
                                out=oht[h][:, :sbn * 16, :],
                                in0=iota16[:].unsqueeze(1).broadcast_to([128, sbn * 16, GSZ]),
                                in1=dstv_sb[:, sc0:sc0 + sbn * 16]
                                    .unsqueeze(2).broadcast_to([128, sbn * 16, GSZ]),
                                op=mybir.AluOpType.is_equal,
                            )
                    if no_mm:
                        continue
                    for si in range(sbn):
                        s = s0 + si                   # dst super 0..NSUP-1
                        for pi in range(2):           # pairs in super
                            pr = s * 2 + pi
                            agg = ppool.tile([128, 128], f32, tag="agg")
                            for gj in range(2):       # groups in pair
                                gi = pi * 2 + gj
                                off = gj * GSZ
                                for hh in range(2):   # src classes
                                    for k in range(CH):
                                        nc.tensor.matmul(
                                            agg[:, off:off + GSZ],
                                            gt[hh][:, si * 16 + gi * CH + k, :],
                                            oht[hh][:, si * 16 + gi * CH + k, :],
                                            start=(hh == 0 and k == 0),
                                            stop=(hh == 1 and k == CH - 1),
                                            skip_group_check=True,
                                        )
                            agg_sb = spool.tile([128, 128], bf16, tag="aggsb")
                            if aggact:
                                nc.scalar.copy(agg_sb[:], agg[:])
                            else:
                                nc.vector.tensor_copy(agg_sb[:], agg[:])
                            hps = hpool.tile([128, D], f32, tag="hps")
                            nc.tensor.matmul(hps[:], agg_sb[:], W_sb[:, l, :],
                                             start=True, stop=True)
                            t_sb = spool.tile([128, D], f32, tag="tsb")
                            nc.vector.scalar_tensor_tensor(
                                out=t_sb[:], in0=hps[:],
                                scalar=ni_sb[:, pr:pr + 1],
                                in1=brep_sb[:, l, :],
                                op0=mybir.AluOpType.mult,
                                op1=mybir.AluOpType.add,
                            )
                            if l < N_LAYERS - 1:
                                # hn = relu(t) * no == relu(t * no), no > 0;
                                # runs on the otherwise-idle ScalarE
                                hn = spool.tile([128, D], bf16, tag="hn2")
                                nc.scalar.activation(
                                    hn[:], t_sb[:],
                                    mybir.ActivationFunctionType.Relu,
                                    scale=no_sb[:, pr:pr + 1],
                                )
                                nc.sync.dma_start(
                                    hn_shard[pr * 128:(pr + 1) * 128, :], hn[:])
                                if pr == P - 1:
                                    ag(hn_shard, l + 1)
                            else:
                                h_sb = spool.tile([128, D], f32, tag="hsb")
                                nc.scalar.activation(
                                    h_sb[:], t_sb[:],
                                    mybir.ActivationFunctionType.Relu,
                                )
                                nc.tensor.matmul(
                                    pool_ps[:], groh[:, pr, :], h_sb[:],
                                    start=(pr == 0), stop=(pr == P - 1),
                                )

            pool_sb = spool.tile([GPC, D], f32, tag="poolsb")
            nc.vector.tensor_scalar_mul(pool_sb[:], pool_ps[:], rc_sb[:])
            nc.sync.dma_start(out_t.ap(), pool_sb[:])

    nc.compile()
    return nc


def make_in_maps(per_core, shared):
    in_maps = []
    for c in range(NCORES):
        pc = per_core[c]
        in_maps.append({
            "idx": pc["idx"], "dstv": pc["dstv"], "ni": pc["ni"],
            "no": pc["no"], "gid": pc["gid"], "rcnt": pc["rcnt"],
            "h0": pc["h0"], "W": shared["W"], "b_rep": shared["b_rep"],
        })
    return in_maps


def kernel(**inputs) -> np.ndarray:
    per_core, shared, meta = preprocess(**inputs)
    nc = build(meta, rep=1)
    in_maps = make_in_maps(per_core, shared)
    res = run_bass_kernel_spmd(nc, in_maps, core_ids=list(range(NCORES)))
    out = np.zeros((N_GRAPHS, D), dtype=np.float32)
    for c in range(NCORES):
        out[meta["core_graphs"][c]] = res.results[c]["out"]
    return out


# revision 44
# speedup vs baseline: 546.4609x; 1.0479x over previous
"""BRPConvEmbedding (3-layer GraphConv + AvgPool readout) on 8 Trainium2 cores.

Sharding: graphs are split contiguously across cores (32 graphs/core), so
pooling is core-local and the output is a pure concat. Each core owns the
nodes of its graphs; nodes are bin-packed into dst-groups of 64 nodes
whose in-degree per src core-half is capped at 512 (4 chunks of 128 edge
slots), making the per-group edge-chunk layout uniform across cores
(single SPMD program).

Per layer: ONE whole-shard AllGather of the bf16 node features; its
rank-major output [8*SH, D] is sliced at row 4*SH into the two
int16-addressable dma_gather bases (no HBM re-layout copy). Edge rows are
fetched with coarse SWDGE dma_gather calls (SB supers per call to
amortize Q7 descriptor generation), the per-edge one-hot is built on
VectorE in the int16 2x mode, the segment-sum runs on TensorE (bf16
lhsT = gathered chunk, rhs = one-hot, PSUM fp32 accumulation), followed
by agg.T @ W (bf16) + fused epilogue and a per-graph pooling matmul.
"""
import numpy as np
from contextlib import ExitStack

import concourse.bacc as bacc
import concourse.mybir as mybir
from concourse import tile
from concourse.bass_utils import run_bass_kernel_spmd

N_NODES = 50000
N_EDGES = 800000
D = 128
N_LAYERS = 3
N_GRAPHS = 256
NCORES = 8
GSZ = 64                  # dst nodes per group
CH = 4                    # chunks per (group, src-class): 4*128 = 512 cap
CAP = CH * 128
GPC = N_GRAPHS // NCORES  # graphs per core
SB = 4                    # supers per dma_gather call

BF16 = None  # set in preprocess via mybir.dt.np


# ----------------------------------------------------------------- host prep
def _pack_once(dA, dB, order):
    """Greedy best-fit packing of nodes into groups of <= GSZ nodes with
    sum(dA) <= CAP and sum(dB) <= CAP per group. Returns group id per node."""
    n = len(dA)
    gids = np.full(n, -1, dtype=np.int64)
    cap = 4 * (-(-n // GSZ)) + 64
    usedA = np.zeros(cap, dtype=np.int64)
    usedB = np.zeros(cap, dtype=np.int64)
    usedN = np.zeros(cap, dtype=np.int64)
    ng = 0
    for i in order:
        a, b = dA[i], dB[i]
        ok = ((usedN[:ng] < GSZ) & (usedA[:ng] + a <= CAP)
              & (usedB[:ng] + b <= CAP))
        if ok.any():
            fit = np.maximum(usedA[:ng] + a, usedB[:ng] + b)
            fit[~ok] = -1
            best = int(np.argmax(fit))
        else:
            best = ng
            ng += 1
        gids[i] = best
        usedA[best] += a
        usedB[best] += b
        usedN[best] += 1
    return gids, ng


def _pack_groups(dA, dB):
    """Multi-start best-fit packing; keeps the attempt with fewest groups."""
    base = np.argsort(-np.maximum(dA, dB), kind="stable")
    best = _pack_once(dA, dB, base)
    for seed in range(2):
        rng = np.random.default_rng(seed)
        key = np.maximum(dA, dB) + rng.uniform(0.0, 1.0, len(dA))
        g, ng = _pack_once(dA, dB, np.argsort(-key, kind="stable"))
        if ng < best[1]:
            best = (g, ng)
    return best


def preprocess(feats, W, b, src, dst, graph_ids):
    bf16 = mybir.dt.np(mybir.dt.bfloat16)
    src = np.asarray(src).astype(np.int64)
    dst = np.asarray(dst).astype(np.int64)
    graph_ids = np.asarray(graph_ids).astype(np.int64)
    feats = np.asarray(feats, dtype=np.float32)

    deg_out = np.maximum(np.bincount(src, minlength=N_NODES), 1).astype(np.float32)
    deg_in = np.maximum(np.bincount(dst, minlength=N_NODES), 1).astype(np.float32)

    # balance graphs across cores by node count (largest-first greedy);
    # kernel() undoes the permutation when assembling the output
    gsizes = np.bincount(graph_ids, minlength=N_GRAPHS)
    core_of_graph = np.zeros(N_GRAPHS, dtype=np.int64)
    loads = np.zeros(NCORES, dtype=np.int64)
    counts = np.zeros(NCORES, dtype=np.int64)
    for g in np.argsort(-gsizes, kind="stable"):
        elig = np.nonzero(counts < GPC)[0]
        c = elig[np.argmin(loads[elig])]
        core_of_graph[g] = c
        loads[c] += gsizes[g]
        counts[c] += 1
    # local graph slot within its core, in ascending global-graph order
    core_graphs = [np.nonzero(core_of_graph == c)[0] for c in range(NCORES)]
    glocal = np.zeros(N_GRAPHS, dtype=np.int64)
    for c in range(NCORES):
        glocal[core_graphs[c]] = np.arange(GPC)

    node_core = core_of_graph[graph_ids]
    core_nodes = [np.nonzero(node_core == c)[0] for c in range(NCORES)]

    # src classes = core halves (the two int16-addressable slices of the
    # whole-shard AllGather output)
    cls = (node_core >= NCORES // 2).astype(np.int64)
    sc = cls[src]
    dA = np.bincount(dst[sc == 0], minlength=N_NODES)
    dB = np.bincount(dst[sc == 1], minlength=N_NODES)

    # pack per core
    packs = {}
    gmax = 0
    for c in range(NCORES):
        n = core_nodes[c]
        g, ng = _pack_groups(dA[n], dB[n])
        packs[c] = (n, g)
        gmax = max(gmax, ng)
    G = -(-gmax // 4) * 4             # groups per shard (mult of 4 = supers)
    SH = G * GSZ                      # rows per core shard
    NSUP = G // 4
    P = G // 2
    R = (NCORES // 2) * SH            # rows per gather base (half the cores)
    assert R <= 32767, f"int16 overflow: {R}"

    # node -> row
    row = np.full(N_NODES, -1, dtype=np.int64)
    for c in range(NCORES):
        n, g = packs[c]
        order = np.lexsort((n, g))
        n_sorted, g_sorted = n[order], g[order]
        slot = np.zeros(len(n), dtype=np.int64)
        _, starts = np.unique(g_sorted, return_index=True)
        for s0, s1 in zip(starts, list(starts[1:]) + [len(n)]):
            slot[s0:s1] = np.arange(s1 - s0)
        row[n_sorted] = c * SH + g_sorted * GSZ + slot

    # gather index into base class q = row within that core-half's block
    gidx = row - cls * R
    assert gidx.min() >= 0 and gidx.max() < R

    # per-core edge layout
    e_core = node_core[dst]
    e_group = (row[dst] - e_core * SH) // GSZ          # 0..G-1 within shard
    e_dslot = (row[dst] - e_core * SH) % GSZ
    e_h = cls[src]
    e_sr = gidx[src]

    per_core = []
    for c in range(NCORES):
        m = np.nonzero(e_core == c)[0]
        g, h, sr, dslt = e_group[m], e_h[m], e_sr[m], e_dslot[m]
        order = np.lexsort((sr, h, g))
        g, h, sr, dslt = g[order], h[order], sr[order], dslt[order]
        key = g * 2 + h
        rank = np.arange(len(m)) - np.searchsorted(key, key, side="left")
        k = rank // 128                                # chunk within (g, h)
        p = rank % 128
        assert (k < CH).all(), "cap exceeded"
        gi = g % 4                                     # group in super
        s = g // 4                                     # super 0..NSUP-1
        c16 = gi * CH + k                              # chunk col in (super, class)
        j = c16 * 128 + p                              # slot in (super, class)

        # idx arrays, class-major: t = h*NSUP + s
        t = h * NSUP + s
        idx16 = np.zeros((2 * NSUP, 16, 128), dtype=np.int16)
        idx16[t, j % 16, j // 16] = sr.astype(np.int16)
        idx_all = np.tile(idx16, (1, 8, 1)).reshape(2 * NSUP, 128, 128)
        idx_2d = idx_all.transpose(1, 0, 2).reshape(128, 2 * NSUP * 128).copy()

        # dst one-hot scalars [128, 2*NSUP*16], -1 for pad slots (int16 so
        # the one-hot is_equal runs in the DVE 16-bit 2x mode)
        dstv = np.full((128, 2 * NSUP * 16), -1, dtype=np.int16)
        dstv[j % 128, t * 16 + c16] = dslt.astype(np.int16)

        # per-pair node scalars [128, P]
        nodes_c = core_nodes[c]
        lr = row[nodes_c] - c * SH
        ni_t = np.ones((128, P), dtype=np.float32)
        no_t = np.ones((128, P), dtype=np.float32)
        gid_t = np.full((128, P), -1.0, dtype=np.float32)
        pr = lr // 128
        pp = lr % 128
        ni_t[pp, pr] = 1.0 / np.sqrt(deg_in[nodes_c])
        no_t[pp, pr] = 1.0 / np.sqrt(deg_out[nodes_c])
        gid_t[pp, pr] = glocal[graph_ids[nodes_c]].astype(np.float32)

        rcnt = (1.0 / np.maximum(
            np.bincount(glocal[graph_ids[nodes_c]], minlength=GPC), 1
        )).astype(np.float32).reshape(GPC, 1)

        # layer-0 input precomputed on host: hn0 = feats * norm_out (bf16)
        h0 = np.zeros((SH, D), dtype=np.float32)
        h0[lr] = feats[nodes_c] / np.sqrt(deg_out[nodes_c])[:, None]
        h0 = h0.astype(bf16)

        per_core.append(dict(
            idx=idx_2d, dstv=dstv, ni=ni_t, no=no_t,
            gid=gid_t, rcnt=rcnt, h0=h0,
        ))

    b_rep = np.broadcast_to(
        np.asarray(b, dtype=np.float32)[None, :, :], (128, N_LAYERS, D)
    ).copy()
    W_bf = np.ascontiguousarray(
        np.asarray(W, dtype=np.float32).transpose(1, 0, 2)
    ).astype(bf16)
    meta = dict(G=G, P=P, NSUP=NSUP, SH=SH, R=R, core_graphs=core_graphs)
    shared = dict(W=W_bf, b_rep=b_rep)
    return per_core, shared, meta


# ------------------------------------------------------------- device build
def build(meta, rep=1, no_coll=False, no_gather=False, sp=False, nq=4,
          no_mm=False, no_oh=False, scratch=65536, sb=SB, gbufs=2,
          aggact=False, pbufs=4, sbufs=4, gp=2):
    G, P, NSUP, SH, R = (meta["G"], meta["P"], meta["NSUP"],
                         meta["SH"], meta["R"])
    # gather blocks: sb supers per call (+ remainder block), 2 classes each
    blocks = [(b * sb, sb) for b in range(NSUP // sb)]
    if NSUP % sb:
        blocks.append((NSUP - NSUP % sb, NSUP % sb))
    f32 = mybir.dt.float32
    bf16 = mybir.dt.bfloat16
    i16 = mybir.dt.int16

    nc = bacc.Bacc("TRN2", target_bir_lowering=False, debug=False,
                   num_devices=NCORES, dynamic_dma_scratch_size=scratch,
                   num_swdge_queues=nq)

    idx_t = nc.dram_tensor("idx", [128, 2 * NSUP * 128], i16, kind="ExternalInput")
    dstv_t = nc.dram_tensor("dstv", [128, 2 * NSUP * 16], i16, kind="ExternalInput")
    ni_t = nc.dram_tensor("ni", [128, P], f32, kind="ExternalInput")
    no_t = nc.dram_tensor("no", [128, P], f32, kind="ExternalInput")
    gid_t = nc.dram_tensor("gid", [128, P], f32, kind="ExternalInput")
    rcnt_t = nc.dram_tensor("rcnt", [GPC, 1], f32, kind="ExternalInput")
    h0_t = nc.dram_tensor("h0", [SH, D], bf16, kind="ExternalInput")
    W_t = nc.dram_tensor("W", [128, N_LAYERS, D], bf16, kind="ExternalInput")
    brep_t = nc.dram_tensor("b_rep", [128, N_LAYERS, D], f32, kind="ExternalInput")
    out_t = nc.dram_tensor("out", [GPC, D], f32, kind="ExternalOutput")

    # Whole-shard AllGather output, double-buffered by layer parity. The
    # rank-major output [8*SH, D] is sliced into two int16-addressable
    # gather bases: rows [0, R) = cores 0-3, rows [R, 2R) = cores 4-7.
    hnall = [
        nc.dram_tensor(f"hnall{par}", [NCORES * SH, D], bf16,
                       kind="Internal", addr_space="Shared")
        for par in range(2)
    ]

    def ag(hn_shard, l):
        """AllGather the full shard into hnall[l % 2]."""
        if no_coll:
            return
        nc.gpsimd.collective_compute(
            "AllGather", mybir.AluOpType.bypass,
            replica_groups=[list(range(NCORES))],
            ins=[hn_shard[:].opt()],
            outs=[hnall[l % 2].ap().opt()],
        )

    with tile.TileContext(nc) as tc, ExitStack() as ctx:
        dram = ctx.enter_context(tc.tile_pool(name="dram", bufs=1, space="DRAM"))
        stat = ctx.enter_context(tc.tile_pool(name="stat", bufs=1))
        gpool = ctx.enter_context(tc.tile_pool(name="gath", bufs=gbufs))
        opool = ctx.enter_context(tc.tile_pool(name="oh", bufs=2))
        spool = ctx.enter_context(tc.tile_pool(name="sb", bufs=sbufs))
        ppool = ctx.enter_context(tc.tile_pool(name="agg_ps", bufs=pbufs, space="PSUM"))
        hpool = ctx.enter_context(tc.tile_pool(name="h_ps", bufs=2, space="PSUM"))
        plpool = ctx.enter_context(tc.tile_pool(name="pool_ps", bufs=1, space="PSUM"))

        hn_shard = dram.tile([SH, D], bf16)

        # ---- statics
        idx_sb = stat.tile([128, 2 * NSUP * 128], i16)
        nc.sync.dma_start(idx_sb[:], idx_t.ap())
        dstv_sb = stat.tile([128, 2 * NSUP * 16], i16)
        nc.sync.dma_start(dstv_sb[:], dstv_t.ap())
        W_sb = stat.tile([128, N_LAYERS, D], bf16)
        nc.sync.dma_start(W_sb[:], W_t.ap())
        brep_sb = stat.tile([128, N_LAYERS, D], f32)
        nc.sync.dma_start(brep_sb[:], brep_t.ap())
        gid_sb = stat.tile([128, P], f32)
        nc.sync.dma_start(gid_sb[:], gid_t.ap())
        rc_sb = stat.tile([GPC, 1], f32)
        nc.sync.dma_start(rc_sb[:], rcnt_t.ap())
        ni_sb = stat.tile([128, P], f32)
        nc.sync.dma_start(ni_sb[:], ni_t.ap())
        no_sb = stat.tile([128, P], f32)
        nc.sync.dma_start(no_sb[:], no_t.ap())

        iota16 = stat.tile([128, GSZ], mybir.dt.int16)
        nc.gpsimd.iota(iota16[:], pattern=[[1, GSZ]], base=0, channel_multiplier=0)
        iota_f = stat.tile([128, GSZ], f32)
        nc.vector.tensor_copy(iota_f[:], iota16[:])

        # graph one-hot [128, P, GPC] (pooling uses layer-2 h)
        groh = stat.tile([128, P, GPC], f32)
        nc.vector.tensor_tensor(
            out=groh[:],
            in0=iota_f[:, :GPC].unsqueeze(1).broadcast_to([128, P, GPC]),
            in1=gid_sb[:].unsqueeze(2).broadcast_to([128, P, GPC]),
            op=mybir.AluOpType.is_equal,
        )

        for _ in range(rep):
            # ---- layer 0 input: hn0 precomputed on host; stage + AllGather
            nc.sync.dma_start(hn_shard[:], h0_t.ap())
            ag(hn_shard, 0)

            pool_ps = plpool.tile([GPC, D], f32)

            for l in range(N_LAYERS):
                par = l % 2
                for bi, (s0, sbn) in enumerate(blocks):
                    gt = [None, None]
                    oht = [None, None]
                    for h in range(2):     # h = src class (gather base)
                        # padded to SB supers so the remainder block shares
                        # the same pool tag (same SBUF slots)
                        gt[h] = gpool.tile([128, sb * 16, D], bf16,
                                           tag=f"g{h}", name=f"g{h}")
                        col0 = (h * NSUP + s0) * 128
                        if not no_gather:
                            # gp pieces per call, spread across SWDGE queues
                            cpp = sbn * 16 // gp          # chunks per piece
                            for pc in range(gp):
                                nc.gpsimd.dma_gather(
                                    out_ap=gt[h][:, pc * cpp:(pc + 1) * cpp, :],
                                    in_ap=hnall[par].ap()[h * R:(h + 1) * R, :],
                                    idxs_ap=idx_sb[:, col0 + pc * cpp * 8:
                                                   col0 + (pc + 1) * cpp * 8],
                                    num_idxs=cpp * 128,
                                    num_idxs_reg=cpp * 128,
                                    elem_size=D, single_packet=sp,
                                    queue_num=(gp * (2 * bi + h) + pc) % nq,
                                )
                        oht[h] = opool.tile([128, sb * 16, GSZ], bf16,
                                            tag=f"oh{h}", name=f"oh{h}")
                        sc0 = (h * NSUP + s0) * 16
                        if not no_oh:
                            nc.vector.tensor_tensor(
                                out=oht[h][:, :sbn * 16, :],
                                in0=iota16[:].unsqueeze(1).broadcast_to([128, sbn * 16, GSZ]),
                                in1=dstv_sb[:, sc0:sc0 + sbn * 16]
                                    .unsqueeze(2).broadcast_to([128, sbn * 16, GSZ]),
                                op=mybir.AluOpType.is_equal,
                            )
                    if no_mm:
                        continue
                    for si in range(sbn):
                        s = s0 + si                   # dst super 0..NSUP-1
                        for pi in range(2):           # pairs in super
                            pr = s * 2 + pi
                            agg = ppool.tile([128, 128], f32, tag="agg")
                            for gj in range(2):       # groups in pair
                                gi = pi * 2 + gj
                                off = gj * GSZ
                                for hh in range(2):   # src classes
                                    for k in range(CH):
                                        nc.tensor.matmul(
                                            agg[:, off:off + GSZ],
                                            gt[hh][:, si * 16 + gi * CH + k, :],
                                            oht[hh][:, si * 16 + gi * CH + k, :],
                                            start=(hh == 0 and k == 0),
                                            stop=(hh == 1 and k == CH - 1),
                                            skip_group_check=True,
                                        )
                            agg_sb = spool.tile([128, 128], bf16, tag="aggsb")
                            if aggact:
                                nc.scalar.copy(agg_sb[:], agg[:])
                            else:
                                nc.vector.tensor_copy(agg_sb[:], agg[:])
                            hps = hpool.tile([128, D], f32, tag="hps")
                            nc.tensor.matmul(hps[:], agg_sb[:], W_sb[:, l, :],
                                             start=True, stop=True)
                            t_sb = spool.tile([128, D], f32, tag="tsb")
                            nc.vector.scalar_tensor_tensor(
                                out=t_sb[:], in0=hps[:],
                                scalar=ni_sb[:, pr:pr + 1],
                                in1=brep_sb[:, l, :],
                                op0=mybir.AluOpType.mult,
                                op1=mybir.AluOpType.add,
                            )
                            if l < N_LAYERS - 1:
                                # hn = relu(t) * no == relu(t * no), no > 0;
                                # runs on the otherwise-idle ScalarE
                                hn = spool.tile([128, D], bf16, tag="hn2")
                                nc.scalar.activation(
                                    hn[:], t_sb[:],
                                    mybir.ActivationFunctionType.Relu,
                                    scale=no_sb[:, pr:pr + 1],
                                )
                                nc.sync.dma_start(
                                    hn_shard[pr * 128:(pr + 1) * 128, :], hn[:])
                                if pr == P - 1:
                                    ag(hn_shard, l + 1)
                            else:
                                h_sb = spool.tile([128, D], f32, tag="hsb")
                                nc.scalar.activation(
                                    h_sb[:], t_sb[:],
                                    mybir.ActivationFunctionType.Relu,
                                )
                                nc.tensor.matmul(
                                    pool_ps[:], groh[:, pr, :], h_sb[:],
                                    start=(pr == 0), stop=(pr == P - 1),
                                )

            pool_sb = spool.tile([GPC, D], f32, tag="poolsb")
            nc.vector.tensor_scalar_mul(pool_sb[:], pool_ps[:], rc_sb[:])
            nc.sync.dma_start(out_t.ap(), pool_sb[:])

    nc.compile()
    return nc


def make_in_maps(per_core, shared):
    in_maps = []
    for c in range(NCORES):
        pc = per_core[c]
        in_maps.append({
            "idx": pc["idx"], "dstv": pc["dstv"], "ni": pc["ni"],
            "no": pc["no"], "gid": pc["gid"], "rcnt": pc["rcnt"],
            "h0": pc["h0"], "W": shared["W"], "b_rep": shared["b_rep"],
        })
    return in_maps


def kernel(**inputs) -> np.ndarray:
    per_core, shared, meta = preprocess(**inputs)
    nc = build(meta, rep=1)
    in_maps = make_in_maps(per_core, shared)
    res = run_bass_kernel_spmd(nc, in_maps, core_ids=list(range(NCORES)))
    out = np.zeros((N_GRAPHS, D), dtype=np.float32)
    for c in range(NCORES):
        out[meta["core_graphs"][c]] = res.results[c]["out"]
    return out


# revision 46
# speedup vs baseline: 806.2828x; 1.4755x over previous
"""BRPConvEmbedding (3-layer GraphConv + AvgPool readout) on 8 Trainium2 cores.

Sharding: graphs are split contiguously across cores (32 graphs/core), so
pooling is core-local and the output is a pure concat. Each core owns the
nodes of its graphs; nodes are bin-packed into dst-groups of 64 nodes
whose in-degree per src core-half is capped at 512 (4 chunks of 128 edge
slots), making the per-group edge-chunk layout uniform across cores
(single SPMD program).

Per layer: ONE whole-shard AllGather of the bf16 node features; its
rank-major output [8*SH, D] is sliced at row 4*SH into the two
int16-addressable dma_gather bases (no HBM re-layout copy). Edge rows are
fetched with coarse SWDGE dma_gather calls (SB supers per call to
amortize Q7 descriptor generation), the per-edge one-hot is built on
VectorE in the int16 2x mode, the segment-sum runs on TensorE (bf16
lhsT = gathered chunk, rhs = one-hot, PSUM fp32 accumulation), followed
by agg.T @ W (bf16) + fused epilogue and a per-graph pooling matmul.
"""
import numpy as np
from contextlib import ExitStack

import concourse.bacc as bacc
import concourse.mybir as mybir
from concourse import tile
from concourse.bass_utils import run_bass_kernel_spmd

N_NODES = 50000
N_EDGES = 800000
D = 128
N_LAYERS = 3
N_GRAPHS = 256
NCORES = 8
GSZ = 64                  # dst nodes per group
CH = 4                    # chunks per (group, src-class): 4*128 = 512 cap
CAP = CH * 128
GPC = N_GRAPHS // NCORES  # graphs per core
SB = 4                    # supers per dma_gather call

BF16 = None  # set in preprocess via mybir.dt.np


# ----------------------------------------------------------------- host prep
def _pack_once(dA, dB, order):
    """Greedy best-fit packing of nodes into groups of <= GSZ nodes with
    sum(dA) <= CAP and sum(dB) <= CAP per group. Returns group id per node."""
    n = len(dA)
    gids = np.full(n, -1, dtype=np.int64)
    cap = 4 * (-(-n // GSZ)) + 64
    usedA = np.zeros(cap, dtype=np.int64)
    usedB = np.zeros(cap, dtype=np.int64)
    usedN = np.zeros(cap, dtype=np.int64)
    ng = 0
    for i in order:
        a, b = dA[i], dB[i]
        ok = ((usedN[:ng] < GSZ) & (usedA[:ng] + a <= CAP)
              & (usedB[:ng] + b <= CAP))
        if ok.any():
            fit = np.maximum(usedA[:ng] + a, usedB[:ng] + b)
            fit[~ok] = -1
            best = int(np.argmax(fit))
        else:
            best = ng
            ng += 1
        gids[i] = best
        usedA[best] += a
        usedB[best] += b
        usedN[best] += 1
    return gids, ng


def _pack_groups(dA, dB):
    """Multi-start best-fit packing; keeps the attempt with fewest groups."""
    base = np.argsort(-np.maximum(dA, dB), kind="stable")
    best = _pack_once(dA, dB, base)
    for seed in range(2):
        rng = np.random.default_rng(seed)
        key = np.maximum(dA, dB) + rng.uniform(0.0, 1.0, len(dA))
        g, ng = _pack_once(dA, dB, np.argsort(-key, kind="stable"))
        if ng < best[1]:
            best = (g, ng)
    return best


def preprocess(feats, W, b, src, dst, graph_ids):
    bf16 = mybir.dt.np(mybir.dt.bfloat16)
    src = np.asarray(src).astype(np.int64)
    dst = np.asarray(dst).astype(np.int64)
    graph_ids = np.asarray(graph_ids).astype(np.int64)
    feats = np.asarray(feats, dtype=np.float32)

    deg_out = np.maximum(np.bincount(src, minlength=N_NODES), 1).astype(np.float32)
    deg_in = np.maximum(np.bincount(dst, minlength=N_NODES), 1).astype(np.float32)

    # balance graphs across cores by node count (largest-first greedy);
    # kernel() undoes the permutation when assembling the output
    gsizes = np.bincount(graph_ids, minlength=N_GRAPHS)
    core_of_graph = np.zeros(N_GRAPHS, dtype=np.int64)
    loads = np.zeros(NCORES, dtype=np.int64)
    counts = np.zeros(NCORES, dtype=np.int64)
    for g in np.argsort(-gsizes, kind="stable"):
        elig = np.nonzero(counts < GPC)[0]
        c = elig[np.argmin(loads[elig])]
        core_of_graph[g] = c
        loads[c] += gsizes[g]
        counts[c] += 1
    # local graph slot within its core, in ascending global-graph order
    core_graphs = [np.nonzero(core_of_graph == c)[0] for c in range(NCORES)]
    glocal = np.zeros(N_GRAPHS, dtype=np.int64)
    for c in range(NCORES):
        glocal[core_graphs[c]] = np.arange(GPC)

    node_core = core_of_graph[graph_ids]
    core_nodes = [np.nonzero(node_core == c)[0] for c in range(NCORES)]

    # src classes = core halves (the two int16-addressable slices of the
    # whole-shard AllGather output)
    cls = (node_core >= NCORES // 2).astype(np.int64)
    sc = cls[src]
    dA = np.bincount(dst[sc == 0], minlength=N_NODES)
    dB = np.bincount(dst[sc == 1], minlength=N_NODES)

    # pack per core
    packs = {}
    gmax = 0
    for c in range(NCORES):
        n = core_nodes[c]
        g, ng = _pack_groups(dA[n], dB[n])
        packs[c] = (n, g)
        gmax = max(gmax, ng)
    G = -(-gmax // 4) * 4             # groups per shard (mult of 4 = supers)
    SH = G * GSZ                      # rows per core shard
    NSUP = G // 4
    P = G // 2
    R = (NCORES // 2) * SH            # rows per gather base (half the cores)
    assert R <= 32767, f"int16 overflow: {R}"

    # node -> row
    row = np.full(N_NODES, -1, dtype=np.int64)
    for c in range(NCORES):
        n, g = packs[c]
        order = np.lexsort((n, g))
        n_sorted, g_sorted = n[order], g[order]
        slot = np.zeros(len(n), dtype=np.int64)
        _, starts = np.unique(g_sorted, return_index=True)
        for s0, s1 in zip(starts, list(starts[1:]) + [len(n)]):
            slot[s0:s1] = np.arange(s1 - s0)
        row[n_sorted] = c * SH + g_sorted * GSZ + slot

    # gather index into base class q = row within that core-half's block
    gidx = row - cls * R
    assert gidx.min() >= 0 and gidx.max() < R

    # per-core edge layout
    e_core = node_core[dst]
    e_group = (row[dst] - e_core * SH) // GSZ          # 0..G-1 within shard
    e_dslot = (row[dst] - e_core * SH) % GSZ
    e_h = cls[src]
    e_sr = gidx[src]

    per_core = []
    for c in range(NCORES):
        m = np.nonzero(e_core == c)[0]
        g, h, sr, dslt = e_group[m], e_h[m], e_sr[m], e_dslot[m]
        order = np.lexsort((sr, h, g))
        g, h, sr, dslt = g[order], h[order], sr[order], dslt[order]
        key = g * 2 + h
        rank = np.arange(len(m)) - np.searchsorted(key, key, side="left")
        k = rank // 128                                # chunk within (g, h)
        p = rank % 128
        assert (k < CH).all(), "cap exceeded"
        gi = g % 4                                     # group in super
        s = g // 4                                     # super 0..NSUP-1
        c16 = gi * CH + k                              # chunk col in (super, class)
        j = c16 * 128 + p                              # slot in (super, class)

        # idx arrays, class-major: t = h*NSUP + s
        t = h * NSUP + s
        idx16 = np.zeros((2 * NSUP, 16, 128), dtype=np.int16)
        idx16[t, j % 16, j // 16] = sr.astype(np.int16)
        idx_all = np.tile(idx16, (1, 8, 1)).reshape(2 * NSUP, 128, 128)
        idx_2d = idx_all.transpose(1, 0, 2).reshape(128, 2 * NSUP * 128).copy()

        # dst one-hot scalars [128, 2*NSUP*16], -1 for pad slots (int16 so
        # the one-hot is_equal runs in the DVE 16-bit 2x mode)
        dstv = np.full((128, 2 * NSUP * 16), -1, dtype=np.int16)
        dstv[j % 128, t * 16 + c16] = dslt.astype(np.int16)

        # per-pair node scalars [128, P]
        nodes_c = core_nodes[c]
        lr = row[nodes_c] - c * SH
        ni_t = np.ones((128, P), dtype=np.float32)
        no_t = np.ones((128, P), dtype=np.float32)
        gid_t = np.full((128, P), -1.0, dtype=np.float32)
        pr = lr // 128
        pp = lr % 128
        ni_t[pp, pr] = 1.0 / np.sqrt(deg_in[nodes_c])
        no_t[pp, pr] = 1.0 / np.sqrt(deg_out[nodes_c])
        gid_t[pp, pr] = glocal[graph_ids[nodes_c]].astype(np.float32)

        rcnt = (1.0 / np.maximum(
            np.bincount(glocal[graph_ids[nodes_c]], minlength=GPC), 1
        )).astype(np.float32).reshape(GPC, 1)

        # layer-0 input precomputed on host: hn0 = feats * norm_out (bf16)
        h0 = np.zeros((SH, D), dtype=np.float32)
        h0[lr] = feats[nodes_c] / np.sqrt(deg_out[nodes_c])[:, None]
        h0 = h0.astype(bf16)

        per_core.append(dict(
            idx=idx_2d, dstv=dstv, ni=ni_t, no=no_t,
            gid=gid_t, rcnt=rcnt, h0=h0,
        ))

    b_rep = np.broadcast_to(
        np.asarray(b, dtype=np.float32)[None, :, :], (128, N_LAYERS, D)
    ).copy()
    W_bf = np.ascontiguousarray(
        np.asarray(W, dtype=np.float32).transpose(1, 0, 2)
    ).astype(bf16)
    meta = dict(G=G, P=P, NSUP=NSUP, SH=SH, R=R, core_graphs=core_graphs)
    shared = dict(W=W_bf, b_rep=b_rep)
    return per_core, shared, meta


# ------------------------------------------------------------- device build
def build(meta, rep=1, no_coll=False, no_gather=False, sp=False, nq=4,
          no_mm=False, no_oh=False, scratch=65536, sb=SB, gbufs=2,
          aggact=False, pbufs=4, sbufs=4, gp=2, obufs=3):
    G, P, NSUP, SH, R = (meta["G"], meta["P"], meta["NSUP"],
                         meta["SH"], meta["R"])
    # gather blocks: sb supers per call (+ remainder block), 2 classes each
    blocks = [(b * sb, sb) for b in range(NSUP // sb)]
    if NSUP % sb:
        blocks.append((NSUP - NSUP % sb, NSUP % sb))
    f32 = mybir.dt.float32
    bf16 = mybir.dt.bfloat16
    i16 = mybir.dt.int16

    nc = bacc.Bacc("TRN2", target_bir_lowering=False, debug=False,
                   num_devices=NCORES, dynamic_dma_scratch_size=scratch,
                   num_swdge_queues=nq)

    idx_t = nc.dram_tensor("idx", [128, 2 * NSUP * 128], i16, kind="ExternalInput")
    dstv_t = nc.dram_tensor("dstv", [128, 2 * NSUP * 16], i16, kind="ExternalInput")
    ni_t = nc.dram_tensor("ni", [128, P], f32, kind="ExternalInput")
    no_t = nc.dram_tensor("no", [128, P], f32, kind="ExternalInput")
    gid_t = nc.dram_tensor("gid", [128, P], f32, kind="ExternalInput")
    rcnt_t = nc.dram_tensor("rcnt", [GPC, 1], f32, kind="ExternalInput")
    h0_t = nc.dram_tensor("h0", [SH, D], bf16, kind="ExternalInput")
    W_t = nc.dram_tensor("W", [128, N_LAYERS, D], bf16, kind="ExternalInput")
    brep_t = nc.dram_tensor("b_rep", [128, N_LAYERS, D], f32, kind="ExternalInput")
    out_t = nc.dram_tensor("out", [GPC, D], f32, kind="ExternalOutput")

    # Whole-shard AllGather output, double-buffered by layer parity. The
    # rank-major output [8*SH, D] is sliced into two int16-addressable
    # gather bases: rows [0, R) = cores 0-3, rows [R, 2R) = cores 4-7.
    hnall = [
        nc.dram_tensor(f"hnall{par}", [NCORES * SH, D], bf16,
                       kind="Internal", addr_space="Shared")
        for par in range(2)
    ]

    def ag(hn_shard, l):
        """AllGather the full shard into hnall[l % 2]."""
        if no_coll:
            return
        nc.gpsimd.collective_compute(
            "AllGather", mybir.AluOpType.bypass,
            replica_groups=[list(range(NCORES))],
            ins=[hn_shard[:].opt()],
            outs=[hnall[l % 2].ap().opt()],
        )

    with tile.TileContext(nc) as tc, ExitStack() as ctx:
        dram = ctx.enter_context(tc.tile_pool(name="dram", bufs=1, space="DRAM"))
        stat = ctx.enter_context(tc.tile_pool(name="stat", bufs=1))
        gpool = ctx.enter_context(tc.tile_pool(name="gath", bufs=gbufs))
        opool = ctx.enter_context(tc.tile_pool(name="oh", bufs=obufs))
        spool = ctx.enter_context(tc.tile_pool(name="sb", bufs=sbufs))
        ppool = ctx.enter_context(tc.tile_pool(name="agg_ps", bufs=pbufs, space="PSUM"))
        hpool = ctx.enter_context(tc.tile_pool(name="h_ps", bufs=2, space="PSUM"))
        plpool = ctx.enter_context(tc.tile_pool(name="pool_ps", bufs=1, space="PSUM"))

        hn_shard = dram.tile([SH, D], bf16)

        # ---- statics
        idx_sb = stat.tile([128, 2 * NSUP * 128], i16)
        nc.sync.dma_start(idx_sb[:], idx_t.ap())
        dstv_sb = stat.tile([128, 2 * NSUP * 16], i16)
        nc.sync.dma_start(dstv_sb[:], dstv_t.ap())
        W_sb = stat.tile([128, N_LAYERS, D], bf16)
        nc.sync.dma_start(W_sb[:], W_t.ap())
        brep_sb = stat.tile([128, N_LAYERS, D], f32)
        nc.sync.dma_start(brep_sb[:], brep_t.ap())
        gid_sb = stat.tile([128, P], f32)
        nc.sync.dma_start(gid_sb[:], gid_t.ap())
        rc_sb = stat.tile([GPC, 1], f32)
        nc.sync.dma_start(rc_sb[:], rcnt_t.ap())
        ni_sb = stat.tile([128, P], f32)
        nc.sync.dma_start(ni_sb[:], ni_t.ap())
        no_sb = stat.tile([128, P], f32)
        nc.sync.dma_start(no_sb[:], no_t.ap())

        iota16 = stat.tile([128, GSZ], mybir.dt.int16)
        nc.gpsimd.iota(iota16[:], pattern=[[1, GSZ]], base=0, channel_multiplier=0)
        iota_f = stat.tile([128, GSZ], f32)
        nc.vector.tensor_copy(iota_f[:], iota16[:])

        # graph one-hot [128, P, GPC] (pooling uses layer-2 h)
        groh = stat.tile([128, P, GPC], f32)
        nc.vector.tensor_tensor(
            out=groh[:],
            in0=iota_f[:, :GPC].unsqueeze(1).broadcast_to([128, P, GPC]),
            in1=gid_sb[:].unsqueeze(2).broadcast_to([128, P, GPC]),
            op=mybir.AluOpType.is_equal,
        )

        for _ in range(rep):
            # ---- layer 0 input: hn0 precomputed on host; stage + AllGather
            nc.sync.dma_start(hn_shard[:], h0_t.ap())
            ag(hn_shard, 0)

            pool_ps = plpool.tile([GPC, D], f32)

            for l in range(N_LAYERS):
                par = l % 2
                for bi, (s0, sbn) in enumerate(blocks):
                    gt = [None, None]
                    oht = [None, None]
                    for h in range(2):     # h = src class (gather base)
                        # padded to SB supers so the remainder block shares
                        # the same pool tag (same SBUF slots)
                        gt[h] = gpool.tile([128, sb * 16, D], bf16,
                                           tag=f"g{h}", name=f"g{h}")
                        col0 = (h * NSUP + s0) * 128
                        if not no_gather:
                            # gp pieces per call, spread across SWDGE queues
                            cpp = sbn * 16 // gp          # chunks per piece
                            for pc in range(gp):
                                nc.gpsimd.dma_gather(
                                    out_ap=gt[h][:, pc * cpp:(pc + 1) * cpp, :],
                                    in_ap=hnall[par].ap()[h * R:(h + 1) * R, :],
                                    idxs_ap=idx_sb[:, col0 + pc * cpp * 8:
                                                   col0 + (pc + 1) * cpp * 8],
                                    num_idxs=cpp * 128,
                                    num_idxs_reg=cpp * 128,
                                    elem_size=D, single_packet=sp,
                                    queue_num=(gp * (2 * bi + h) + pc) % nq,
                                )
                        oht[h] = opool.tile([128, sb * 16, GSZ], bf16,
                                            tag=f"oh{h}", name=f"oh{h}")
                        sc0 = (h * NSUP + s0) * 16
                        if not no_oh:
                            nc.vector.tensor_tensor(
                                out=oht[h][:, :sbn * 16, :],
                                in0=iota16[:].unsqueeze(1).broadcast_to([128, sbn * 16, GSZ]),
                                in1=dstv_sb[:, sc0:sc0 + sbn * 16]
                                    .unsqueeze(2).broadcast_to([128, sbn * 16, GSZ]),
                                op=mybir.AluOpType.is_equal,
                            )
                    if no_mm:
                        continue
                    for si in range(sbn):
                        s = s0 + si                   # dst super 0..NSUP-1
                        for pi in range(2):           # pairs in super
                            pr = s * 2 + pi
                            agg = ppool.tile([128, 128], f32, tag="agg")
                            for gj in range(2):       # groups in pair
                                gi = pi * 2 + gj
                                off = gj * GSZ
                                for hh in range(2):   # src classes
                                    for k in range(CH):
                                        nc.tensor.matmul(
                                            agg[:, off:off + GSZ],
                                            gt[hh][:, si * 16 + gi * CH + k, :],
                                            oht[hh][:, si * 16 + gi * CH + k, :],
                                            start=(hh == 0 and k == 0),
                                            stop=(hh == 1 and k == CH - 1),
                                            skip_group_check=True,
                                        )
                            agg_sb = spool.tile([128, 128], bf16, tag="aggsb")
                            if aggact:
                                nc.scalar.copy(agg_sb[:], agg[:])
                            else:
                                nc.vector.tensor_copy(agg_sb[:], agg[:])
                            hps = hpool.tile([128, D], f32, tag="hps")
                            nc.tensor.matmul(hps[:], agg_sb[:], W_sb[:, l, :],
                                             start=True, stop=True)
                            t_sb = spool.tile([128, D], f32, tag="tsb")
                            nc.vector.scalar_tensor_tensor(
                                out=t_sb[:], in0=hps[:],
                                scalar=ni_sb[:, pr:pr + 1],
                                in1=brep_sb[:, l, :],
                                op0=mybir.AluOpType.mult,
                                op1=mybir.AluOpType.add,
                            )
                            if l < N_LAYERS - 1:
                                # hn = relu(t) * no == relu(t * no), no > 0;
                                # runs on the otherwise-idle ScalarE
                                hn = spool.tile([128, D], bf16, tag="hn2")
                                nc.scalar.activation(
                                    hn[:], t_sb[:],
                                    mybir.ActivationFunctionType.Relu,
                                    scale=no_sb[:, pr:pr + 1],
                                )
                                nc.sync.dma_start(
                                    hn_shard[pr * 128:(pr + 1) * 128, :], hn[:])
                                if pr == P - 1:
                                    ag(hn_shard, l + 1)
                            else:
                                h_sb = spool.tile([128, D], f32, tag="hsb")
                                nc.scalar.activation(
                                    h_sb[:], t_sb[:],
                                    mybir.ActivationFunctionType.Relu,
                                )
                                nc.tensor.matmul(
                                    pool_ps[:], groh[:, pr, :], h_sb[:],
                                    start=(pr == 0), stop=(pr == P - 1),
                                )

            pool_sb = spool.tile([GPC, D], f32, tag="poolsb")
            nc.vector.tensor_scalar_mul(pool_sb[:], pool_ps[:], rc_sb[:])
            nc.sync.dma_start(out_t.ap(), pool_sb[:])

    nc.compile()
    return nc


def make_in_maps(per_core, shared):
    in_maps = []
    for c in range(NCORES):
        pc = per_core[c]
        in_maps.append({
            "idx": pc["idx"], "dstv": pc["dstv"], "ni": pc["ni"],
            "no": pc["no"], "gid": pc["gid"], "rcnt": pc["rcnt"],
            "h0": pc["h0"], "W": shared["W"], "b_rep": shared["b_rep"],
        })
    return in_maps


def kernel(**inputs) -> np.ndarray:
    per_core, shared, meta = preprocess(**inputs)
    nc = build(meta, rep=1)
    in_maps = make_in_maps(per_core, shared)
    res = run_bass_kernel_spmd(nc, in_maps, core_ids=list(range(NCORES)))
    out = np.zeros((N_GRAPHS, D), dtype=np.float32)
    for c in range(NCORES):
        out[meta["core_graphs"][c]] = res.results[c]["out"]
    return out
